# revision 32
# baseline (speedup 1.0000x reference)
"""Trainium2 Bass kernel for a GQA attention block (RMSNorm -> QKV+gate ->
Q/K-norm -> RoPE -> attention -> gated out -> proj), tensor-parallel over
heads across 8 NeuronCores.

Sharding: core c owns q heads [5c, 5c+5) and kv group c (NQ=40, NKV=8).
Each core computes a partial projection output; partials are summed on host
(row-parallel proj unshard).

Perf structure (v2):
  - fp16 inputs/probabilities/v/o/out (f32 PSUM accumulation), q/k in f32r.
  - QKV phase split into two psum pass-groups per 512-col chunk so drains
    overlap matmuls (PE never waits on psum banks).
  - norms/rope/stats/v-transposes for chunk c emitted during chunk c+1 so
    the attention phase starts with everything roped and runs heads
    back-to-back on the PE.
  - softmax exp has a -ln(16) bias folded in (cancels in the ratio) so
    fp16 denominators can't overflow.
  - deferred per-head output scaling (gate * 1/denom) executes during the
    next head; projection in fp16 with prefetched weights.
"""
import contextlib
import math
import sys

sys.path.insert(0, "/opt/trn_rl_repo")

import numpy as np

import bass_rust as _bass_rust

import concourse.bacc as bacc
import concourse.tile as tile
from concourse import mybir
from concourse.hw_specs import get_activation_tables


class _Bacc(bacc.Bacc):
    """Bacc with activation-table choice restricted to the exp+ln set.

    The stock insert_act_table_loads pass picks the FIRST act_func_set
    containing each function, so alternating Ln/Exp activations thrash
    between two tables.  Emptying all sets except `natural_log_exp_and_others`
    (square/copy/exp/ln) and `sigmoid_and_others` forces one resident table
    for the whole kernel (plus a single swap around the sigmoid).
    """

    _KEEP_SETS = {"natural_log_exp_and_others", "sigmoid_and_others"}

    def insert_act_table_loads(self):
        has_activation = any(
            isinstance(i, mybir.InstActivation)
            for b in self.main_func.blocks
            for i in b.instructions
        )
        if not has_activation:
            return
        tables = [
            (name, (fns if name in self._KEEP_SETS else set()))
            for name, fns in get_activation_tables(self.m.arch).items()
        ]
        _bass_rust.insert_act_table_loads(self, tables)

NQ, NKV, D, HID = 40, 8, 128, 5120
S = 2048
NC = 8
HPC = NQ // NC          # q heads per core = 5
EPS = 1e-6
HT = HID // 128         # 40 hid tiles
ST = S // 128           # 16 seq tiles of 128
NCH = S // 512          # 4 chunks of 512
KT = S // 128           # 16 k-tiles
QKV_COLS = HPC * D + 2 * D + HPC   # 901
NLN16 = -math.log(16.0)            # exp bias so fp16 sums can't overflow
F32 = mybir.dt.float32
F32R = mybir.dt.float32r
F16 = mybir.dt.float16
BF16 = mybir.dt.bfloat16
AF = mybir.ActivationFunctionType
BUILD_OPTS = {}


def build_program(repeat=1):
    opt = BUILD_OPTS
    nc = _Bacc(None, target_bir_lowering=False)

    # register activation-bias constants (mirrors Bass.__init__ registration)
    for val in (EPS, float(D) * EPS, NLN16):
        t = nc.alloc_sbuf_tensor(f"const-float32-{val}", [128, 1], F32)
        nc.gpsimd.memset(t.ap(), val)
        nc.const_aps.aps[(F32, val)] = t.ap()
    nc.all_engine_barrier()

    # ---- I/O ----
    xT = nc.dram_tensor("xT", [HT, 128, S], F16, kind="ExternalInput")
    wq = nc.dram_tensor("wq", [HT, 128, QKV_COLS], F16, kind="ExternalInput")
    wp = nc.dram_tensor("wp", [HPC, 128, HID], BF16, kind="ExternalInput")
    cosq = nc.dram_tensor("cosq", [128, S], F16, kind="ExternalInput")
    sinq = nc.dram_tensor("sinq", [128, S], F16, kind="ExternalInput")
    cosk = nc.dram_tensor("cosk", [128, S], F16, kind="ExternalInput")
    sink = nc.dram_tensor("sink", [128, S], F16, kind="ExternalInput")
    ones_col = nc.dram_tensor("ones_col", [128, 1], F32R, kind="ExternalInput")
    ident = nc.dram_tensor("ident", [128, 128], F32R, kind="ExternalInput")
    out = nc.dram_tensor("out", [S, HID], BF16, kind="ExternalOutput")
    dbg = {}
    if opt.get("debug"):
        dbg["q0"] = nc.dram_tensor("dbg_q0", [128, S], F32, kind="ExternalOutput")
        dbg["k"] = nc.dram_tensor("dbg_k", [128, S], F32, kind="ExternalOutput")
        dbg["vnat"] = nc.dram_tensor("dbg_vnat", [128, S], BF16,
                                     kind="ExternalOutput")
        dbg["nk"] = nc.dram_tensor("dbg_nk", [128, KT], F32,
                                   kind="ExternalOutput")
        dbg["gates"] = nc.dram_tensor("dbg_gates", [HPC, S], F32,
                                      kind="ExternalOutput")
        dbg["scale"] = nc.dram_tensor("dbg_scale", [HPC, S], F32,
                                      kind="ExternalOutput")
        dbg["o0"] = nc.dram_tensor("dbg_o0", [128, S], BF16,
                                   kind="ExternalOutput")

    with tile.TileContext(nc, pool_alloc_mode=opt.get("palloc", "stack")) as tc:
      for _rep in range(repeat):
        with tc.tile_pool(name=f"persist{_rep}", bufs=1) as pers, \
             tc.tile_pool(name=f"cols{_rep}", bufs=1) as cols, \
             tc.tile_pool(name=f"scr{_rep}", bufs=1, space="DRAM") as dscr:
            # DRAM row bounces (for partition-broadcast / row->col reshape)
            lnm_scr = dscr.tile([1, S], F32, name="lnm_scr")
            rrow_scr = dscr.tile([1, S], F32, name="rrow_scr")
            lnk_scr = dscr.tile([1, S], F32, name="lnk_scr")
            gate_scr = dscr.tile([HPC, S], F32, name="gate_scr")
            nq_scr = dscr.tile([HPC, S], F32, name="nq_scr")
            scale_scr = dscr.tile([HPC, S], F32, name="scale_scr")
            # persistent small tiles
            t_ones = cols.tile([128, 1], F32R, name="ones")
            nc.sync.dma_start(t_ones[:, :], ones_col[:, :])
            t_ones16 = cols.tile([128, 1], F16, name="ones16")
            nc.gpsimd.memset(t_ones16[:, :], 1.0)
            t_onesb = cols.tile([128, 1], BF16, name="onesb")
            nc.gpsimd.memset(t_onesb[:, :], 1.0)
            t_id = cols.tile([128, 128], F32R, name="ident")
            nc.sync.dma_start(t_id[:, :], ident[:, :])
            # rope tables resident (fp16)
            tcq = cols.tile([128, S], F16, name="tcq")
            tsq = cols.tile([128, S], F16, name="tsq")
            tck = cols.tile([128, S], F16, name="tck")
            tsk = cols.tile([128, S], F16, name="tsk")
            nc.sync.dma_start(tcq[:, :], cosq[:, :])
            nc.sync.dma_start(tsq[:, :], sinq[:, :])
            nc.sync.dma_start(tck[:, :], cosk[:, :])
            nc.sync.dma_start(tsk[:, :], sink[:, :])

            q_t = [pers.tile([128, S], F32R, name=f"q{h}") for h in range(HPC)]
            k_t = pers.tile([128, S], F32R, name="kT")
            vnat = pers.tile([128, S], BF16, name="vnat")
            o_t = [pers.tile([128, S], BF16, name=f"o{h}") for h in range(HPC)]
            gates = pers.tile([HPC, S], F32, name="gates")
            r_col = cols.tile([128, KT], F32, name="r_col")
            nk_col = cols.tile([128, KT], F32, name="nk_col")
            # proj ntp0 weights, prefetched during attention
            wt0 = pers.tile([128, HPC, 1024], BF16, name="wt0")

            # ============ Phase 1: QKV (+ fused stats/rope/transposes) =====
            NQD = 10  # hid quad-tiles per chunk pass
            with contextlib.ExitStack() as _ph1:
                ent = _ph1.enter_context
                psA = ent(tc.tile_pool(name=f"psA{_rep}", bufs=1, space="PSUM"))
                psB = ent(tc.tile_pool(name=f"psB{_rep}", bufs=1, space="PSUM"))
                psS = ent(tc.tile_pool(name=f"psS{_rep}", bufs=1, space="PSUM"))
                xtp = ent(tc.tile_pool(name=f"xt{_rep}", bufs=3))
                wsap = ent(tc.tile_pool(name=f"wsa{_rep}", bufs=3))
                wsbp = ent(tc.tile_pool(name=f"wsb{_rep}", bufs=2))
                sqbig = ent(tc.tile_pool(name=f"sq{_rep}", bufs=2))
                sqsml = ent(tc.tile_pool(name=f"sqs{_rep}", bufs=2))
                accp = ent(tc.tile_pool(name=f"accx{_rep}", bufs=1))
                accrp = ent(tc.tile_pool(name=f"accr{_rep}", bufs=2))
                vstp = ent(tc.tile_pool(name=f"vst{_rep}", bufs=2))
                rowp = ent(tc.tile_pool(name=f"row{_rep}", bufs=2))
                ropep = ent(tc.tile_pool(name=f"rope{_rep}", bufs=2))

                def emit_stats(ch):
                    """Partition-sum stats for chunk ch (runs during ch+1).

                    Sequential through the psS bank: pre-norm row, k row,
                    q rows.  Emits the DRAM bounces the rope/gate/exp-scale
                    consumers read back.
                    """
                    c0 = ch * 512
                    # pre-norm: lnm = ln(mean_hid x^2 + eps); accr folded by
                    # the chunk-ch pass-B code into rowp tile (returned there)
                    accr = stats_accr.pop(ch)
                    pr = psS.tile([1, 512], F32, name="sm")
                    nc.tensor.matmul(pr[:, :], t_ones16[:, :], accr[:, :],
                                     start=True, stop=True)
                    lnm_row = rowp.tile([1, 512], F32, name="lnm_row")
                    nc.scalar.activation(lnm_row[:, :], pr[:, :], AF.Ln,
                                         bias=EPS, scale=1.0 / HID)
                    nc.sync.dma_start(lnm_scr[0:1, c0:c0 + 512], lnm_row[:, :])
                    r_row = rowp.tile([1, 512], F32, name="r_row")
                    nc.scalar.activation(r_row[:, :], lnm_row[:, :], AF.Exp,
                                         bias=0.0, scale=-0.5)
                    nc.sync.dma_start(rrow_scr[0:1, c0:c0 + 512], r_row[:, :])
                    # r as columns for the v fold
                    lcp = rowp.tile([128, 4], F32, name="lcp")
                    nc.sync.dma_start(
                        lcp[:, :],
                        lnm_scr[0, c0:c0 + 512].rearrange("(t p) -> p t", p=128))
                    nc.scalar.activation(r_col[:, ch * 4:ch * 4 + 4], lcp[:, :],
                                         AF.Exp, bias=0.0, scale=-0.5)
                    # k-norm stats
                    ksq = sqsml.tile([128, 512], F16, name="sqc")
                    nc.scalar.activation(ksq[:, :],
                                         k_t[:, c0:c0 + 512].bitcast(F32),
                                         AF.Square)
                    pn = psS.tile([1, 512], F32, name="sm")
                    nc.tensor.matmul(pn[:, :], t_ones16[:, :], ksq[:, :],
                                     start=True, stop=True)
                    lnk_row = rowp.tile([1, 512], F32, name="lnk_row")
                    nc.scalar.activation(lnk_row[:, :], pn[:, :], AF.Ln,
                                         bias=D * EPS, scale=1.0)
                    nc.sync.dma_start(lnk_scr[0:1, c0:c0 + 512], lnk_row[:, :])
                    # q-norm stats per head -> nq rows in DRAM
                    for h in range(HPC):
                        qsq = sqsml.tile([128, 512], F16, name="sqc")
                        nc.scalar.activation(qsq[:, :],
                                             q_t[h][:, c0:c0 + 512].bitcast(F32),
                                             AF.Square)
                        pq = psS.tile([1, 512], F32, name="sm")
                        nc.tensor.matmul(pq[:, :], t_ones16[:, :], qsq[:, :],
                                         start=True, stop=True)
                        lnq = rowp.tile([1, 512], F32, name="lnq")
                        nc.scalar.activation(lnq[:, :], pq[:, :], AF.Ln,
                                             bias=EPS, scale=1.0 / D)
                        nqr = rowp.tile([1, 512], F32, name="nqr")
                        nc.scalar.activation(nqr[:, :], lnq[:, :], AF.Exp,
                                             bias=0.0, scale=-0.5)
                        nc.sync.dma_start(nq_scr[h:h + 1, c0:c0 + 512],
                                          nqr[:, :])

                def emit_vtrans(ch):
                    """v transposes for chunk ch -> vnat (fp16, r folded)."""
                    vstg = v_stg.pop(ch)
                    for j in range(4):
                        kt = ch * 4 + j
                        ptr = psS.tile([128, 128], F32R, name="sm")
                        nc.tensor.transpose(ptr[:, :],
                                            vstg[:, j * 128:(j + 1) * 128],
                                            t_id[:, :])
                        nc.vector.tensor_scalar_mul(
                            vnat[:, kt * 128:(kt + 1) * 128], ptr[:, :],
                            r_col[:, kt:kt + 1])

                def emit_rope(ch, heads=None, do_k=True):
                    """RoPE (+q-norm fold) for chunk ch, in place."""
                    c0 = ch * 512
                    if do_k:
                        rot = ropep.tile([128, 512], F32, name="rot")
                        nc.sync.dma_start(rot[0:64, :],
                                          k_t[64:128, c0:c0 + 512].bitcast(F32))
                        nc.sync.dma_start(rot[64:128, :],
                                          k_t[0:64, c0:c0 + 512].bitcast(F32))
                        t1 = ropep.tile([128, 512], F32, name="t1")
                        nc.vector.tensor_mul(t1[:, :],
                                             k_t[:, c0:c0 + 512].bitcast(F32),
                                             tck[:, c0:c0 + 512])
                        nc.vector.tensor_mul(rot[:, :], rot[:, :],
                                             tsk[:, c0:c0 + 512])
                        nc.vector.tensor_add(k_t[:, c0:c0 + 512], t1[:, :],
                                             rot[:, :])
                    for h in (range(HPC) if heads is None else heads):
                        nb = ropep.tile([128, 512], F32, name="nb")
                        nc.sync.dma_start(
                            nb[:, :],
                            nq_scr[h:h + 1, c0:c0 + 512].to_broadcast((128, 512)))
                        rot = ropep.tile([128, 512], F32, name="rot")
                        nc.sync.dma_start(rot[0:64, :],
                                          q_t[h][64:128, c0:c0 + 512].bitcast(F32))
                        nc.sync.dma_start(rot[64:128, :],
                                          q_t[h][0:64, c0:c0 + 512].bitcast(F32))
                        t1 = ropep.tile([128, 512], F32, name="t1")
                        nc.vector.tensor_mul(t1[:, :],
                                             q_t[h][:, c0:c0 + 512].bitcast(F32),
                                             tcq[:, c0:c0 + 512])
                        nc.vector.tensor_mul(rot[:, :], rot[:, :],
                                             tsq[:, c0:c0 + 512])
                        nc.vector.tensor_add(t1[:, :], t1[:, :], rot[:, :])
                        nc.vector.tensor_mul(q_t[h][:, c0:c0 + 512], t1[:, :],
                                             nb[:, :])

                stats_accr = {}
                v_stg = {}
                for ch in range(NCH):
                    c0 = ch * 512
                    # ---------------- pass A: q heads 0..3 ----------------
                    pmA = [psA.tile([128, 512], F32, name=f"a{m}")
                           for m in range(4)]
                    accx = accp.tile([128, 2048], F16, name="accx")
                    for hq in range(NQD):
                        ht = 4 * hq
                        xt = xtp.tile([128, 4, 512], F16, name="xt")
                        nc.sync.dma_start(
                            xt[:, :, :],
                            xT[ht:ht + 4, :, c0:c0 + 512].rearrange(
                                "t p c -> p t c"))
                        ws = wsap.tile([128, 4, 512], F16, name="wsa")
                        nc.sync.dma_start(
                            ws[:, :, :],
                            wq[ht:ht + 4, :, 0:512].rearrange("t p c -> p t c"))
                        for i in range(4):
                            for m in range(4):
                                nc.tensor.matmul(
                                    pmA[m][:, :], ws[:, i, m * 128:(m + 1) * 128],
                                    xt[:, i, :], start=(hq == 0 and i == 0),
                                    stop=(hq == NQD - 1 and i == 3))
                        # x^2 accumulation for the pre-norm (fp16, gpsimd)
                        sq = sqbig.tile([128, 2048], F16, name="sq")
                        nc.scalar.activation(
                            sq[:, :],
                            xt[:, :, :].rearrange("p t c -> p (t c)"),
                            AF.Square)
                        if hq == 0:
                            nc.gpsimd.tensor_copy(accx[:, :], sq[:, :])
                        else:
                            nc.gpsimd.tensor_add(accx[:, :], accx[:, :],
                                                 sq[:, :])
                        # interleave previous chunk's post-work between quads
                        if ch > 0:
                            if hq == 1:
                                emit_stats(ch - 1)
                            elif hq == 4:
                                emit_vtrans(ch - 1)
                            elif hq == 6:
                                emit_rope(ch - 1)
                    # ---------------- pass B: q4, k, v, gate --------------
                    pmB = [psB.tile([128, 512], F32, name=f"b{m}")
                           for m in range(3)]
                    pg = psS.tile([5, 512], F32, name="sm")
                    for hq in range(NQD):
                        ht = 4 * hq
                        xt2 = xtp.tile([128, 4, 512], F16, name="xt")
                        nc.sync.dma_start(
                            xt2[:, :, :],
                            xT[ht:ht + 4, :, c0:c0 + 512].rearrange(
                                "t p c -> p t c"))
                        ws = wsbp.tile([128, 4, 389], F16, name="wsb")
                        nc.sync.dma_start(
                            ws[:, :, :],
                            wq[ht:ht + 4, :, 512:901].rearrange("t p c -> p t c"))
                        for i in range(4):
                            for m in range(3):
                                nc.tensor.matmul(
                                    pmB[m][:, :], ws[:, i, m * 128:(m + 1) * 128],
                                    xt2[:, i, :], start=(hq == 0 and i == 0),
                                    stop=(hq == NQD - 1 and i == 3))
                            nc.tensor.matmul(pg[:, :], ws[:, i, 384:389],
                                             xt2[:, i, :],
                                             start=(hq == 0 and i == 0),
                                             stop=(hq == NQD - 1 and i == 3))
                        if hq == 0:
                            # drains of pass A overlap pass-B matmuls
                            nc.vector.tensor_copy(q_t[0][:, c0:c0 + 512],
                                                  pmA[0][:, :])
                            nc.scalar.copy(q_t[1][:, c0:c0 + 512], pmA[1][:, :])
                        elif hq == 1:
                            nc.vector.tensor_copy(q_t[2][:, c0:c0 + 512],
                                                  pmA[2][:, :])
                            nc.scalar.copy(q_t[3][:, c0:c0 + 512], pmA[3][:, :])
                        elif hq == 2:
                            # fold x^2 quad-halves -> accr for emit_stats
                            accr = accrp.tile([128, 512], F16, name="accr")
                            nc.vector.tensor_add(accr[:, :], accx[:, 0:512],
                                                 accx[:, 512:1024])
                            nc.vector.tensor_add(accr[:, :], accr[:, :],
                                                 accx[:, 1024:1536])
                            nc.vector.tensor_add(accr[:, :], accr[:, :],
                                                 accx[:, 1536:2048])
                            stats_accr[ch] = accr
                    # drains of pass B (overlap next chunk's pass A)
                    nc.scalar.copy(q_t[4][:, c0:c0 + 512], pmB[0][:, :])
                    nc.vector.tensor_copy(k_t[:, c0:c0 + 512], pmB[1][:, :])
                    vstg = vstp.tile([128, 512], F32R, name="vstg")
                    nc.vector.tensor_copy(vstg[:, :], pmB[2][:, :])
                    v_stg[ch] = vstg
                    nc.vector.tensor_copy(gates[:, c0:c0 + 512], pg[:, :])

                # tail: post-work for the last chunk; k + q0 first so
                # attention head 0 can start as early as possible
                emit_stats(NCH - 1)
                emit_vtrans(NCH - 1)
                emit_rope(NCH - 1, heads=[0], do_k=True)
                # gates: g = sigmoid(r * g_raw) (one table swap)
                for ch in range(NCH):
                    c0 = ch * 512
                    rb = rowp.tile([5, 512], F32, name="rb")
                    nc.sync.dma_start(
                        rb[:, :],
                        rrow_scr[0:1, c0:c0 + 512].to_broadcast((5, 512)))
                    nc.vector.tensor_mul(gates[:, c0:c0 + 512],
                                         gates[:, c0:c0 + 512], rb[:, :])
                nc.scalar.activation(gates[:, :], gates[:, :], AF.Sigmoid)
                nc.sync.dma_start(gate_scr[:, :], gates[:, :])
                # exp-scale columns for the attention softmax
                lnk_col = rowp.tile([128, KT], F32, name="lnk_col")
                nc.sync.dma_start(lnk_col[:, :],
                                  lnk_scr[0, :].rearrange("(t p) -> p t", p=128))
                nc.scalar.activation(nk_col[:, :], lnk_col[:, :], AF.Exp,
                                     bias=0.0, scale=-0.5)
                emit_rope(NCH - 1, heads=[1, 2, 3, 4], do_k=False)
                # prefetch proj ntp0 weights (lands during attention)
                nc.sync.dma_start(
                    wt0[:, :, :],
                    wp[:, :, 0:1024].rearrange("h p c -> p h c"))

            # ============ Phase 2: attention ============================
            with contextlib.ExitStack() as _ph2:
                ent = _ph2.enter_context
                pSC = ent(tc.tile_pool(name=f"at_sc{_rep}", bufs=2, space="PSUM"))
                pAV = ent(tc.tile_pool(name=f"at_av{_rep}", bufs=1, space="PSUM"))
                pMisc = ent(tc.tile_pool(name=f"at_ms{_rep}", bufs=2, space="PSUM"))
                etp = ent(tc.tile_pool(name=f"at_et{_rep}", bufs=3))
                acp = ent(tc.tile_pool(name=f"at_ac{_rep}", bufs=2))
                asb = ent(tc.tile_pool(name=f"at_sb{_rep}", bufs=2))
                for h in range(HPC):
                    den = [asb.tile([1, 512], F32, name=f"den{i}")
                           for i in range(4)]
                    for qp in range(2):
                        c0 = qp * 1024
                        po = [pAV.tile([128, 512], F32, name=f"av{j}")
                              for j in range(2)]
                        # 4 short denominator chains (bounds bf16 rounding):
                        # 0,1 on vector; 2,3 on gpsimd
                        acc = [acp.tile([128, 1024], BF16, name=f"acc{i}")
                               for i in range(4)]
                        ps_tiles = {}

                        def emit_sc(kt):
                            k0 = kt * 128
                            ps = pSC.tile([128, 1024], F32, name="sc")
                            for j in range(2):
                                nc.tensor.matmul(
                                    ps[:, j * 512:(j + 1) * 512],
                                    k_t[:, k0:k0 + 128],
                                    q_t[h][:, c0 + j * 512:c0 + (j + 1) * 512],
                                    start=True, stop=True)
                            ps_tiles[kt] = ps

                        emit_sc(0)
                        for kt in range(KT):
                            k0 = kt * 128
                            if kt + 1 < KT:
                                emit_sc(kt + 1)
                            ps = ps_tiles.pop(kt)
                            et = etp.tile([128, 1024], BF16, name="et")
                            nc.scalar.activation(et[:, :], ps[:, :], AF.Exp,
                                                 bias=NLN16,
                                                 scale=nk_col[:, kt:kt + 1])
                            for j in range(2):
                                nc.tensor.matmul(po[j][:, :],
                                                 vnat[:, k0:k0 + 128],
                                                 et[:, j * 512:(j + 1) * 512],
                                                 start=(kt == 0),
                                                 stop=(kt == KT - 1))
                            a = kt % 4
                            eng = nc.vector if a < 2 else nc.gpsimd
                            if kt < 4:
                                eng.tensor_copy(acc[a][:, :], et[:, :])
                            else:
                                eng.tensor_add(acc[a][:, :], acc[a][:, :],
                                               et[:, :])
                        for j in range(2):
                            s0 = c0 + j * 512
                            srow = pMisc.tile([1, 512], F32, name="srow")
                            for a in range(4):
                                nc.tensor.matmul(srow[:, :], t_onesb[:, :],
                                                 acc[a][:, j * 512:(j + 1) * 512],
                                                 start=(a == 0), stop=(a == 3))
                            nc.scalar.copy(den[qp * 2 + j][:, :], srow[:, :])
                            # drain AV psum (unscaled; scaled next head)
                            if j == 0:
                                nc.scalar.copy(o_t[h][:, s0:s0 + 512],
                                               po[j][:, :])
                            else:
                                nc.vector.tensor_copy(o_t[h][:, s0:s0 + 512],
                                                      po[j][:, :])
                    # deferred scale: rcp = gate / den, bounced via DRAM,
                    # applied to o_t[h] while head h+1 computes
                    for blk in range(4):
                        s0 = blk * 512
                        rcp = asb.tile([1, 512], F32, name=f"rcp{blk}")
                        nc.vector.reciprocal(rcp[:, :], den[blk][:, :])
                        grow = asb.tile([1, 512], F32, name=f"grow{blk}")
                        nc.sync.dma_start(grow[:, :],
                                          gate_scr[h:h + 1, s0:s0 + 512])
                        nc.vector.tensor_mul(rcp[:, :], rcp[:, :], grow[:, :])
                        nc.sync.dma_start(scale_scr[h:h + 1, s0:s0 + 512],
                                          rcp[:, :])
                        sb = asb.tile([128, 512], F32, name="scaleb")
                        nc.sync.dma_start(
                            sb[:, :],
                            scale_scr[h:h + 1, s0:s0 + 512].to_broadcast(
                                (128, 512)))
                        nc.vector.tensor_mul(o_t[h][:, s0:s0 + 512],
                                             o_t[h][:, s0:s0 + 512], sb[:, :])

            if opt.get("debug"):
                nc.sync.dma_start(dbg["q0"][:, :], q_t[0][:, :].bitcast(F32))
                nc.sync.dma_start(dbg["k"][:, :], k_t[:, :].bitcast(F32))
                nc.sync.dma_start(dbg["vnat"][:, :], vnat[:, :])
                nc.sync.dma_start(dbg["nk"][:, :], nk_col[:, :])
                nc.sync.dma_start(dbg["gates"][:, :], gates[:, :])
                nc.sync.dma_start(dbg["scale"][:, :], scale_scr[:, :])
                nc.sync.dma_start(dbg["o0"][:, :], o_t[0][:, :])

            # ============ Phase 3: projection ===========================
            with contextlib.ExitStack() as _ph3:
                ent = _ph3.enter_context
                pPJ = ent(tc.tile_pool(name=f"pj_ps{_rep}", bufs=4, space="PSUM"))
                pjs = ent(tc.tile_pool(name=f"pj_sb{_rep}", bufs=2))
                pjw = ent(tc.tile_pool(name=f"pj_wt{_rep}", bufs=2))
                NTP = HID // 1024
                wt_sets = {0: wt0}

                def emit_wt(ntp):
                    n0 = ntp * 1024
                    wt = pjw.tile([128, HPC, 1024], BF16, name="wt")
                    nc.sync.dma_start(
                        wt[:, :, :],
                        wp[:, :, n0:n0 + 1024].rearrange("h p c -> p h c"))
                    wt_sets[ntp] = wt

                for ntp in range(NTP):
                    n0 = ntp * 1024
                    if ntp + 1 < NTP:
                        emit_wt(ntp + 1)
                    wt = wt_sets.pop(ntp)
                    for st in range(ST):
                        s0 = st * 128
                        ob = pjs.tile([128, 1024], BF16, name="outsb")
                        pp = [pPJ.tile([128, 512], F32, name=f"pj{j}")
                              for j in range(2)]
                        for j in range(2):
                            for h in range(HPC):
                                nc.tensor.matmul(
                                    pp[j][:, :], o_t[h][:, s0:s0 + 128],
                                    wt[:, h, j * 512:(j + 1) * 512],
                                    start=(h == 0), stop=(h == HPC - 1))
                        nc.vector.tensor_copy(ob[:, 0:512], pp[0][:, :])
                        nc.scalar.copy(ob[:, 512:1024], pp[1][:, :])
                        nc.sync.dma_start(out[s0:s0 + 128, n0:n0 + 1024],
                                          ob[:, :])
    nc.finalize()
    return nc


# ---------------- host-side prep & execution ----------------

_CACHE = {}


def _get_exec(repeat=1):
    key = (repeat, tuple(sorted(BUILD_OPTS.items())))
    if key in _CACHE:
        return _CACHE[key]

    import jax
    from concourse import bass2jax, mybir as mb
    from jax.experimental.shard_map import shard_map
    from jax.sharding import Mesh, PartitionSpec

    bass2jax.install_neuronx_cc_hook()
    nc = build_program(repeat)

    part_name = nc.partition_id_tensor.name if nc.partition_id_tensor else None
    in_names, out_names, out_avals = [], [], []
    for alloc in nc.m.functions[0].allocations:
        if not isinstance(alloc, mb.MemoryLocationSet):
            continue
        name = alloc.memorylocations[0].name
        if alloc.kind == "ExternalInput":
            if name != part_name:
                in_names.append(name)
        elif alloc.kind == "ExternalOutput":
            out_names.append(name)
            out_avals.append(jax.core.ShapedArray(tuple(alloc.tensor_shape),
                                                  mb.dt.np(alloc.dtype)))
    n_params = len(in_names)
    all_names = in_names + out_names
    if part_name is not None:
        all_names = all_names + [part_name]

    def _body(*args):
        operands = list(args)
        if part_name is not None:
            operands.append(bass2jax.partition_id_tensor())
        outs = bass2jax._bass_exec_p.bind(
            *operands,
            out_avals=tuple(out_avals),
            in_names=tuple(all_names),
            out_names=tuple(out_names),
            lowering_input_output_aliases=(),
            sim_require_finite=True,
            sim_require_nnan=True,
            nc=nc,
        )
        return tuple(outs)

    devices = jax.devices()[:NC]
    mesh = Mesh(np.asarray(devices), ("core",))
    spec = (PartitionSpec("core"),) * (n_params + len(out_names))
    fn = jax.jit(shard_map(_body, mesh=mesh, in_specs=spec,
                           out_specs=(PartitionSpec("core"),) * len(out_names),
                           check_rep=False), keep_unused=True)
    _CACHE[key] = dict(fn=fn, nc=nc, in_names=in_names, out_names=out_names,
                       out_avals=out_avals, mesh=mesh)
    return _CACHE[key]


def prep_inputs(x, rope_cos, rope_sin, w_pre_norm, w_qkv, w_q_norm, w_k_norm,
                w_proj):
    """Build the per-core input dict list (host-side sharding/layout only)."""
    import ml_dtypes
    bf16 = ml_dtypes.bfloat16
    x = np.asarray(x, np.float32)
    w_qkv = np.asarray(w_qkv, np.float32)
    w_proj = np.asarray(w_proj, np.float32)
    w_pre = np.asarray(w_pre_norm, np.float32)
    w_qn = np.asarray(w_q_norm, np.float32)
    w_kn = np.asarray(w_k_norm, np.float32)
    cos = np.asarray(rope_cos, np.float32)[0]   # [S, D]
    sin = np.asarray(rope_sin, np.float32)[0]

    xT = np.ascontiguousarray(x[0].T).reshape(HT, 128, S).astype(np.float16)

    cosT = np.ascontiguousarray(cos.T)          # [D, S]
    sinT = np.ascontiguousarray(sin.T)
    sign = np.where(np.arange(D) < D // 2, -1.0, 1.0).astype(np.float32)

    def rope_tables(w):
        w_swap = np.concatenate([w[D // 2:], w[:D // 2]])
        c = cosT * w[:, None]
        s = sinT * (sign * w_swap)[:, None]
        return (np.ascontiguousarray(c).astype(np.float16),
                np.ascontiguousarray(s).astype(np.float16))

    cq, sq_ = rope_tables(w_qn)
    ck, sk = rope_tables(w_kn)

    wqkv_eff = w_pre[:, None] * w_qkv           # fold pre-norm weight (exact)
    q_dim, k_dim = NQ * D, NKV * D
    ones = np.ones((128, 1), np.float32)
    ident = np.eye(128, dtype=np.float32)

    in_maps = []
    for c in range(NC):
        wslice = np.concatenate([
            wqkv_eff[:, (HPC * c) * D:(HPC * c + HPC) * D],
            wqkv_eff[:, q_dim + c * D:q_dim + (c + 1) * D],
            wqkv_eff[:, q_dim + k_dim + c * D:q_dim + k_dim + (c + 1) * D],
            wqkv_eff[:, q_dim + 2 * k_dim + HPC * c:q_dim + 2 * k_dim + HPC * (c + 1)],
        ], axis=1)                               # [HID, 901]
        wslice = np.ascontiguousarray(wslice).reshape(
            HT, 128, QKV_COLS).astype(np.float16)
        wpslice = np.ascontiguousarray(
            w_proj[(HPC * c) * D:(HPC * c + HPC) * D, :]).reshape(
            HPC, 128, HID).astype(bf16)
        in_maps.append({
            "xT": xT, "wq": wslice, "wp": wpslice,
            "cosq": cq, "sinq": sq_, "cosk": ck, "sink": sk,
            "ones_col": ones, "ident": ident,
        })
    return in_maps


def run_in_maps(in_maps):
    """Execute the SPMD program; returns list of per-core {out: [S, HID]}."""
    cache = _get_exec()
    fn, in_names, out_names, out_avals = (cache["fn"], cache["in_names"],
                                          cache["out_names"], cache["out_avals"])
    concat_in = [np.concatenate([m[nm] for m in in_maps], axis=0)
                 for nm in in_names]
    zeros = [np.zeros((NC * a.shape[0], *a.shape[1:]), a.dtype) for a in out_avals]
    outs = fn(*concat_in, *zeros)
    res = []
    for c in range(NC):
        d = {}
        for i, nm in enumerate(out_names):
            shp = out_avals[i].shape
            d[nm] = np.asarray(outs[i]).reshape(NC, *shp)[c]
        res.append(d)
    return res


def kernel(**inputs):
    in_maps = prep_inputs(**inputs)
    res = run_in_maps(in_maps)
    total = res[0]["out"].astype(np.float32)
    for c in range(1, NC):
        total = total + res[c]["out"].astype(np.float32)
    return total.reshape(1, S, HID)


# revision 40
# speedup vs baseline: 1.0125x; 1.0125x over previous
"""Trainium2 Bass kernel for a GQA attention block (RMSNorm -> QKV+gate ->
Q/K-norm -> RoPE -> attention -> gated out -> proj), tensor-parallel over
heads across 8 NeuronCores.

Sharding: core c owns q heads [5c, 5c+5) and kv group c (NQ=40, NKV=8).
Each core computes a partial projection output; partials are summed on host
(row-parallel proj unshard).

Perf structure (v2):
  - fp16 inputs/probabilities/v/o/out (f32 PSUM accumulation), q/k in f32r.
  - QKV phase split into two psum pass-groups per 512-col chunk so drains
    overlap matmuls (PE never waits on psum banks).
  - norms/rope/stats/v-transposes for chunk c emitted during chunk c+1 so
    the attention phase starts with everything roped and runs heads
    back-to-back on the PE.
  - softmax exp has a -ln(16) bias folded in (cancels in the ratio) so
    fp16 denominators can't overflow.
  - deferred per-head output scaling (gate * 1/denom) executes during the
    next head; projection in fp16 with prefetched weights.
"""
import contextlib
import math
import sys

sys.path.insert(0, "/opt/trn_rl_repo")

import numpy as np

import bass_rust as _bass_rust

import concourse.bacc as bacc
import concourse.tile as tile
from concourse import mybir
from concourse.hw_specs import get_activation_tables


class _Bacc(bacc.Bacc):
    """Bacc with activation-table choice restricted to the exp+ln set.

    The stock insert_act_table_loads pass picks the FIRST act_func_set
    containing each function, so alternating Ln/Exp activations thrash
    between two tables.  Emptying all sets except `natural_log_exp_and_others`
    (square/copy/exp/ln) and `sigmoid_and_others` forces one resident table
    for the whole kernel (plus a single swap around the sigmoid).
    """

    _KEEP_SETS = {"natural_log_exp_and_others", "sigmoid_and_others"}

    def insert_act_table_loads(self):
        has_activation = any(
            isinstance(i, mybir.InstActivation)
            for b in self.main_func.blocks
            for i in b.instructions
        )
        if not has_activation:
            return
        tables = [
            (name, (fns if name in self._KEEP_SETS else set()))
            for name, fns in get_activation_tables(self.m.arch).items()
        ]
        _bass_rust.insert_act_table_loads(self, tables)

NQ, NKV, D, HID = 40, 8, 128, 5120
S = 2048
NC = 8
HPC = NQ // NC          # q heads per core = 5
EPS = 1e-6
HT = HID // 128         # 40 hid tiles
ST = S // 128           # 16 seq tiles of 128
NCH = S // 512          # 4 chunks of 512
KT = S // 128           # 16 k-tiles
QKV_COLS = HPC * D + 2 * D + HPC   # 901
NLN16 = -math.log(16.0)            # exp bias so fp16 sums can't overflow
F32 = mybir.dt.float32
F32R = mybir.dt.float32r
F16 = mybir.dt.float16
BF16 = mybir.dt.bfloat16
AF = mybir.ActivationFunctionType
BUILD_OPTS = {}


def build_program(repeat=1):
    opt = BUILD_OPTS
    nc = _Bacc(None, target_bir_lowering=False)

    # register activation-bias constants (mirrors Bass.__init__ registration)
    for val in (EPS, float(D) * EPS, NLN16):
        t = nc.alloc_sbuf_tensor(f"const-float32-{val}", [128, 1], F32)
        nc.gpsimd.memset(t.ap(), val)
        nc.const_aps.aps[(F32, val)] = t.ap()
    nc.all_engine_barrier()

    # ---- I/O ----
    xT = nc.dram_tensor("xT", [HT, 128, S], F16, kind="ExternalInput")
    wq = nc.dram_tensor("wq", [HT, 128, QKV_COLS], F16, kind="ExternalInput")
    wp = nc.dram_tensor("wp", [HPC, 128, HID], BF16, kind="ExternalInput")
    cosq = nc.dram_tensor("cosq", [128, S], F16, kind="ExternalInput")
    sinq = nc.dram_tensor("sinq", [128, S], F16, kind="ExternalInput")
    cosk = nc.dram_tensor("cosk", [128, S], F16, kind="ExternalInput")
    sink = nc.dram_tensor("sink", [128, S], F16, kind="ExternalInput")
    ones_col = nc.dram_tensor("ones_col", [128, 1], F32R, kind="ExternalInput")
    ident = nc.dram_tensor("ident", [128, 128], F32R, kind="ExternalInput")
    out = nc.dram_tensor("out", [S, HID], BF16, kind="ExternalOutput")
    dbg = {}
    if opt.get("debug"):
        dbg["q0"] = nc.dram_tensor("dbg_q0", [128, S], F32, kind="ExternalOutput")
        dbg["k"] = nc.dram_tensor("dbg_k", [128, S], F32, kind="ExternalOutput")
        dbg["vnat"] = nc.dram_tensor("dbg_vnat", [128, S], BF16,
                                     kind="ExternalOutput")
        dbg["nk"] = nc.dram_tensor("dbg_nk", [128, KT], F32,
                                   kind="ExternalOutput")
        dbg["gates"] = nc.dram_tensor("dbg_gates", [HPC, S], F32,
                                      kind="ExternalOutput")
        dbg["scale"] = nc.dram_tensor("dbg_scale", [HPC, S], F32,
                                      kind="ExternalOutput")
        dbg["o0"] = nc.dram_tensor("dbg_o0", [128, S], BF16,
                                   kind="ExternalOutput")

    with tile.TileContext(nc, pool_alloc_mode=opt.get("palloc", "stack")) as tc:
      for _rep in range(repeat):
        with tc.tile_pool(name=f"persist{_rep}", bufs=1) as pers, \
             tc.tile_pool(name=f"cols{_rep}", bufs=1) as cols, \
             tc.tile_pool(name=f"scr{_rep}", bufs=1, space="DRAM") as dscr:
            # DRAM row bounces (for partition-broadcast / row->col reshape)
            lnm_scr = dscr.tile([1, S], F32, name="lnm_scr")
            den_scr = dscr.tile([HPC * 4, 512], F32, name="den_scr")
            rrow_scr = dscr.tile([1, S], F32, name="rrow_scr")
            lnk_scr = dscr.tile([1, S], F32, name="lnk_scr")
            gate_scr = dscr.tile([HPC, S], F32, name="gate_scr")
            nq_scr = dscr.tile([HPC, S], F32, name="nq_scr")
            scale_scr = dscr.tile([HPC, S], F32, name="scale_scr")
            # persistent small tiles
            t_ones = cols.tile([128, 1], F32R, name="ones")
            nc.sync.dma_start(t_ones[:, :], ones_col[:, :])
            t_ones16 = cols.tile([128, 1], F16, name="ones16")
            nc.gpsimd.memset(t_ones16[:, :], 1.0)
            t_onesb = cols.tile([128, 1], BF16, name="onesb")
            nc.gpsimd.memset(t_onesb[:, :], 1.0)
            t_id = cols.tile([128, 128], F32R, name="ident")
            nc.sync.dma_start(t_id[:, :], ident[:, :])
            # rope tables resident (fp16); DMAs are emitted after chunk 0's
            # first loads (tables are first needed during chunk 1)
            tcq = cols.tile([128, S], F16, name="tcq")
            tsq = cols.tile([128, S], F16, name="tsq")
            tck = cols.tile([128, S], F16, name="tck")
            tsk = cols.tile([128, S], F16, name="tsk")

            q_t = [pers.tile([128, S], F32R, name=f"q{h}") for h in range(HPC)]
            k_t = pers.tile([128, S], F32R, name="kT")
            vnat = pers.tile([128, S], BF16, name="vnat")
            o_t = [pers.tile([128, S], BF16, name=f"o{h}") for h in range(HPC)]
            gates = pers.tile([HPC, S], F32, name="gates")
            r_col = cols.tile([128, KT], F32, name="r_col")
            nk_col = cols.tile([128, KT], F32, name="nk_col")
            # proj ntp0 weights, prefetched during attention
            wt0 = pers.tile([128, HPC, 1024], BF16, name="wt0")

            # ============ Phase 1: QKV (+ fused stats/rope/transposes) =====
            NQD = 10  # hid quad-tiles per chunk pass
            with contextlib.ExitStack() as _ph1:
                ent = _ph1.enter_context
                psA = ent(tc.tile_pool(name=f"psA{_rep}", bufs=1, space="PSUM"))
                psB = ent(tc.tile_pool(name=f"psB{_rep}", bufs=1, space="PSUM"))
                psS = ent(tc.tile_pool(name=f"psS{_rep}", bufs=1, space="PSUM"))
                xtp = ent(tc.tile_pool(name=f"xt{_rep}", bufs=3))
                xtpb = ent(tc.tile_pool(name=f"xtb{_rep}", bufs=3))
                wsap = ent(tc.tile_pool(name=f"wsa{_rep}", bufs=3))
                wsbp = ent(tc.tile_pool(name=f"wsb{_rep}", bufs=2))
                sqbig = ent(tc.tile_pool(name=f"sq{_rep}", bufs=2))
                sqsml = ent(tc.tile_pool(name=f"sqs{_rep}", bufs=2))
                accp = ent(tc.tile_pool(name=f"accx{_rep}", bufs=1))
                accrp = ent(tc.tile_pool(name=f"accr{_rep}", bufs=2))
                vstp = ent(tc.tile_pool(name=f"vst{_rep}", bufs=2))
                rowp = ent(tc.tile_pool(name=f"row{_rep}", bufs=2))
                ropep = ent(tc.tile_pool(name=f"rope{_rep}", bufs=2))

                def emit_stats(ch):
                    """Partition-sum stats for chunk ch (runs during ch+1).

                    Sequential through the psS bank: pre-norm row, k row,
                    q rows.  Emits the DRAM bounces the rope/gate/exp-scale
                    consumers read back.
                    """
                    c0 = ch * 512
                    # pre-norm: lnm = ln(mean_hid x^2 + eps); accr folded by
                    # the chunk-ch pass-B code into rowp tile (returned there)
                    accr = stats_accr.pop(ch)
                    pr = psS.tile([1, 512], F32, name="sm")
                    nc.tensor.matmul(pr[:, :], t_ones16[:, :], accr[:, :],
                                     start=True, stop=True)
                    lnm_row = rowp.tile([1, 512], F32, name="lnm_row")
                    nc.scalar.activation(lnm_row[:, :], pr[:, :], AF.Ln,
                                         bias=EPS, scale=1.0 / HID)
                    nc.sync.dma_start(lnm_scr[0:1, c0:c0 + 512], lnm_row[:, :])
                    r_row = rowp.tile([1, 512], F32, name="r_row")
                    nc.scalar.activation(r_row[:, :], lnm_row[:, :], AF.Exp,
                                         bias=0.0, scale=-0.5)
                    nc.sync.dma_start(rrow_scr[0:1, c0:c0 + 512], r_row[:, :])
                    # r as columns for the v fold
                    lcp = rowp.tile([128, 4], F32, name="lcp")
                    nc.sync.dma_start(
                        lcp[:, :],
                        lnm_scr[0, c0:c0 + 512].rearrange("(t p) -> p t", p=128))
                    nc.scalar.activation(r_col[:, ch * 4:ch * 4 + 4], lcp[:, :],
                                         AF.Exp, bias=0.0, scale=-0.5)
                    # k-norm stats
                    ksq = sqsml.tile([128, 512], F16, name="sqc")
                    nc.scalar.activation(ksq[:, :],
                                         k_t[:, c0:c0 + 512].bitcast(F32),
                                         AF.Square)
                    pn = psS.tile([1, 512], F32, name="sm")
                    nc.tensor.matmul(pn[:, :], t_ones16[:, :], ksq[:, :],
                                     start=True, stop=True)
                    lnk_row = rowp.tile([1, 512], F32, name="lnk_row")
                    nc.scalar.activation(lnk_row[:, :], pn[:, :], AF.Ln,
                                         bias=D * EPS, scale=1.0)
                    nc.sync.dma_start(lnk_scr[0:1, c0:c0 + 512], lnk_row[:, :])
                    # q-norm stats per head -> nq rows in DRAM
                    for h in range(HPC):
                        qsq = sqsml.tile([128, 512], F16, name="sqc")
                        nc.scalar.activation(qsq[:, :],
                                             q_t[h][:, c0:c0 + 512].bitcast(F32),
                                             AF.Square)
                        pq = psS.tile([1, 512], F32, name="sm")
                        nc.tensor.matmul(pq[:, :], t_ones16[:, :], qsq[:, :],
                                         start=True, stop=True)
                        lnq = rowp.tile([1, 512], F32, name="lnq")
                        nc.scalar.activation(lnq[:, :], pq[:, :], AF.Ln,
                                             bias=EPS, scale=1.0 / D)
                        nqr = rowp.tile([1, 512], F32, name="nqr")
                        nc.scalar.activation(nqr[:, :], lnq[:, :], AF.Exp,
                                             bias=0.0, scale=-0.5)
                        nc.sync.dma_start(nq_scr[h:h + 1, c0:c0 + 512],
                                          nqr[:, :])

                def emit_vtrans(ch):
                    """v transposes for chunk ch -> vnat (fp16, r folded)."""
                    vstg = v_stg.pop(ch)
                    for j in range(4):
                        kt = ch * 4 + j
                        ptr = psS.tile([128, 128], F32R, name="sm")
                        nc.tensor.transpose(ptr[:, :],
                                            vstg[:, j * 128:(j + 1) * 128],
                                            t_id[:, :])
                        nc.vector.tensor_scalar_mul(
                            vnat[:, kt * 128:(kt + 1) * 128], ptr[:, :],
                            r_col[:, kt:kt + 1])

                def emit_rope(ch, heads=None, do_k=True):
                    """RoPE (+q-norm fold) for chunk ch, in place."""
                    c0 = ch * 512
                    if do_k:
                        rot = ropep.tile([128, 512], F32, name="rot")
                        nc.sync.dma_start(rot[0:64, :],
                                          k_t[64:128, c0:c0 + 512].bitcast(F32))
                        nc.sync.dma_start(rot[64:128, :],
                                          k_t[0:64, c0:c0 + 512].bitcast(F32))
                        t1 = ropep.tile([128, 512], F32, name="t1")
                        nc.vector.tensor_mul(t1[:, :],
                                             k_t[:, c0:c0 + 512].bitcast(F32),
                                             tck[:, c0:c0 + 512])
                        nc.vector.tensor_mul(rot[:, :], rot[:, :],
                                             tsk[:, c0:c0 + 512])
                        nc.vector.tensor_add(k_t[:, c0:c0 + 512], t1[:, :],
                                             rot[:, :])
                    for h in (range(HPC) if heads is None else heads):
                        nb = ropep.tile([128, 512], F32, name="nb")
                        nc.sync.dma_start(
                            nb[:, :],
                            nq_scr[h:h + 1, c0:c0 + 512].to_broadcast((128, 512)))
                        rot = ropep.tile([128, 512], F32, name="rot")
                        nc.sync.dma_start(rot[0:64, :],
                                          q_t[h][64:128, c0:c0 + 512].bitcast(F32))
                        nc.sync.dma_start(rot[64:128, :],
                                          q_t[h][0:64, c0:c0 + 512].bitcast(F32))
                        t1 = ropep.tile([128, 512], F32, name="t1")
                        nc.vector.tensor_mul(t1[:, :],
                                             q_t[h][:, c0:c0 + 512].bitcast(F32),
                                             tcq[:, c0:c0 + 512])
                        nc.vector.tensor_mul(rot[:, :], rot[:, :],
                                             tsq[:, c0:c0 + 512])
                        nc.vector.tensor_add(t1[:, :], t1[:, :], rot[:, :])
                        nc.vector.tensor_mul(q_t[h][:, c0:c0 + 512], t1[:, :],
                                             nb[:, :])

                stats_accr = {}
                v_stg = {}
                for ch in range(NCH):
                    c0 = ch * 512
                    # ---------------- pass A: q heads 0..3 ----------------
                    pmA = [psA.tile([128, 512], F32, name=f"a{m}")
                           for m in range(4)]
                    accx = accp.tile([128, 2048], F16, name="accx")
                    for hq in range(NQD):
                        ht = 4 * hq
                        xt = xtp.tile([128, 4, 512], F16, name="xt")
                        nc.sync.dma_start(
                            xt[:, :, :],
                            xT[ht:ht + 4, :, c0:c0 + 512].rearrange(
                                "t p c -> p t c"))
                        ws = wsap.tile([128, 4, 512], F16, name="wsa")
                        nc.sync.dma_start(
                            ws[:, :, :],
                            wq[ht:ht + 4, :, 0:512].rearrange("t p c -> p t c"))
                        for i in range(4):
                            for m in range(4):
                                nc.tensor.matmul(
                                    pmA[m][:, :], ws[:, i, m * 128:(m + 1) * 128],
                                    xt[:, i, :], start=(hq == 0 and i == 0),
                                    stop=(hq == NQD - 1 and i == 3))
                        # x^2 accumulation for the pre-norm (fp16, gpsimd)
                        sq = sqbig.tile([128, 2048], F16, name="sq")
                        nc.scalar.activation(
                            sq[:, :],
                            xt[:, :, :].rearrange("p t c -> p (t c)"),
                            AF.Square)
                        if hq == 0:
                            nc.gpsimd.tensor_copy(accx[:, :], sq[:, :])
                        else:
                            nc.gpsimd.tensor_add(accx[:, :], accx[:, :],
                                                 sq[:, :])
                        # interleave previous chunk's post-work between quads
                        if ch == 0:
                            if hq == 1:
                                nc.sync.dma_start(tcq[:, :], cosq[:, :])
                                nc.sync.dma_start(tsq[:, :], sinq[:, :])
                            elif hq == 2:
                                nc.sync.dma_start(tck[:, :], cosk[:, :])
                                nc.sync.dma_start(tsk[:, :], sink[:, :])
                        if ch > 0:
                            if hq == 1:
                                emit_stats(ch - 1)
                            elif hq == 4:
                                emit_vtrans(ch - 1)
                            elif hq == 6:
                                emit_rope(ch - 1)
                    # ---------------- pass B: q4, k, v, gate --------------
                    pmB = [psB.tile([128, 512], F32, name=f"b{m}")
                           for m in range(3)]
                    pg = psS.tile([5, 512], F32, name="sm")
                    for hq in range(NQD):
                        ht = 4 * hq
                        xt2 = xtpb.tile([128, 4, 512], F16, name="xt")
                        nc.sync.dma_start(
                            xt2[:, :, :],
                            xT[ht:ht + 4, :, c0:c0 + 512].rearrange(
                                "t p c -> p t c"))
                        ws = wsbp.tile([128, 4, 389], F16, name="wsb")
                        nc.sync.dma_start(
                            ws[:, :, :],
                            wq[ht:ht + 4, :, 512:901].rearrange("t p c -> p t c"))
                        for i in range(4):
                            for m in range(3):
                                nc.tensor.matmul(
                                    pmB[m][:, :], ws[:, i, m * 128:(m + 1) * 128],
                                    xt2[:, i, :], start=(hq == 0 and i == 0),
                                    stop=(hq == NQD - 1 and i == 3))
                            nc.tensor.matmul(pg[:, :], ws[:, i, 384:389],
                                             xt2[:, i, :],
                                             start=(hq == 0 and i == 0),
                                             stop=(hq == NQD - 1 and i == 3))
                        if hq == 0:
                            # drains of pass A overlap pass-B matmuls
                            nc.vector.tensor_copy(q_t[0][:, c0:c0 + 512],
                                                  pmA[0][:, :])
                            nc.scalar.copy(q_t[1][:, c0:c0 + 512], pmA[1][:, :])
                        elif hq == 1:
                            nc.vector.tensor_copy(q_t[2][:, c0:c0 + 512],
                                                  pmA[2][:, :])
                            nc.scalar.copy(q_t[3][:, c0:c0 + 512], pmA[3][:, :])
                        elif hq == 2:
                            # fold x^2 quad-halves -> accr for emit_stats
                            accr = accrp.tile([128, 512], F16, name="accr")
                            nc.vector.tensor_add(accr[:, :], accx[:, 0:512],
                                                 accx[:, 512:1024])
                            nc.vector.tensor_add(accr[:, :], accr[:, :],
                                                 accx[:, 1024:1536])
                            nc.vector.tensor_add(accr[:, :], accr[:, :],
                                                 accx[:, 1536:2048])
                            stats_accr[ch] = accr
                    # drains of pass B (overlap next chunk's pass A)
                    nc.scalar.copy(q_t[4][:, c0:c0 + 512], pmB[0][:, :])
                    nc.vector.tensor_copy(k_t[:, c0:c0 + 512], pmB[1][:, :])
                    vstg = vstp.tile([128, 512], F32R, name="vstg")
                    nc.vector.tensor_copy(vstg[:, :], pmB[2][:, :])
                    v_stg[ch] = vstg
                    nc.vector.tensor_copy(gates[:, c0:c0 + 512], pg[:, :])

                # tail: post-work for the last chunk; k + q0 first so
                # attention head 0 can start as early as possible
                emit_stats(NCH - 1)
                emit_vtrans(NCH - 1)
                emit_rope(NCH - 1, heads=[0], do_k=True)
                # gates: g = sigmoid(r * g_raw) (one table swap)
                for ch in range(NCH):
                    c0 = ch * 512
                    rb = rowp.tile([5, 512], F32, name="rb")
                    nc.sync.dma_start(
                        rb[:, :],
                        rrow_scr[0:1, c0:c0 + 512].to_broadcast((5, 512)))
                    nc.vector.tensor_mul(gates[:, c0:c0 + 512],
                                         gates[:, c0:c0 + 512], rb[:, :])
                nc.scalar.activation(gates[:, :], gates[:, :], AF.Sigmoid)
                nc.sync.dma_start(gate_scr[:, :], gates[:, :])
                # exp-scale columns for the attention softmax
                lnk_col = rowp.tile([128, KT], F32, name="lnk_col")
                nc.sync.dma_start(lnk_col[:, :],
                                  lnk_scr[0, :].rearrange("(t p) -> p t", p=128))
                nc.scalar.activation(nk_col[:, :], lnk_col[:, :], AF.Exp,
                                     bias=0.0, scale=-0.5)
                emit_rope(NCH - 1, heads=[1, 2, 3, 4], do_k=False)
                # prefetch proj ntp0 weights (lands during attention)
                nc.sync.dma_start(
                    wt0[:, :, :],
                    wp[:, :, 0:1024].rearrange("h p c -> p h c"))

            # ============ Phase 2: attention ============================
            with contextlib.ExitStack() as _ph2:
                ent = _ph2.enter_context
                pSC = ent(tc.tile_pool(name=f"at_sc{_rep}", bufs=2, space="PSUM"))
                pAV = ent(tc.tile_pool(name=f"at_av{_rep}", bufs=1, space="PSUM"))
                pMisc = ent(tc.tile_pool(name=f"at_ms{_rep}", bufs=2, space="PSUM"))
                etp = ent(tc.tile_pool(name=f"at_et{_rep}", bufs=4))
                acp = ent(tc.tile_pool(name=f"at_ac{_rep}", bufs=2))
                asb = ent(tc.tile_pool(name=f"at_sb{_rep}", bufs=2))
                for h in range(HPC):
                    den = [asb.tile([1, 512], F32, name=f"den{i}")
                           for i in range(4)]
                    for qp in range(2):
                        c0 = qp * 1024
                        po = [pAV.tile([128, 512], F32, name=f"av{j}")
                              for j in range(2)]
                        # 4 short denominator chains (bounds bf16 rounding):
                        # 0,1 on vector; 2,3 on gpsimd
                        acc = [acp.tile([128, 1024], BF16, name=f"acc{i}")
                               for i in range(4)]
                        ps_tiles = {}

                        def emit_sc(kt):
                            k0 = kt * 128
                            ps = pSC.tile([128, 1024], F32, name="sc")
                            for j in range(2):
                                nc.tensor.matmul(
                                    ps[:, j * 512:(j + 1) * 512],
                                    k_t[:, k0:k0 + 128],
                                    q_t[h][:, c0 + j * 512:c0 + (j + 1) * 512],
                                    start=True, stop=True)
                            ps_tiles[kt] = ps

                        emit_sc(0)
                        for kt in range(KT):
                            k0 = kt * 128
                            if kt + 1 < KT:
                                emit_sc(kt + 1)
                            ps = ps_tiles.pop(kt)
                            et = etp.tile([128, 1024], BF16, name="et")
                            nc.scalar.activation(et[:, :], ps[:, :], AF.Exp,
                                                 bias=NLN16,
                                                 scale=nk_col[:, kt:kt + 1])
                            for j in range(2):
                                nc.tensor.matmul(po[j][:, :],
                                                 vnat[:, k0:k0 + 128],
                                                 et[:, j * 512:(j + 1) * 512],
                                                 start=(kt == 0),
                                                 stop=(kt == KT - 1))
                            a = kt % 4
                            if kt < 4:
                                nc.vector.tensor_copy(acc[a][:, :], et[:, :])
                            else:
                                nc.vector.tensor_add(acc[a][:, :],
                                                     acc[a][:, :], et[:, :])
                        for j in range(2):
                            s0 = c0 + j * 512
                            srow = pMisc.tile([1, 512], F32, name="srow")
                            for a in range(4):
                                nc.tensor.matmul(srow[:, :], t_onesb[:, :],
                                                 acc[a][:, j * 512:(j + 1) * 512],
                                                 start=(a == 0), stop=(a == 3))
                            nc.scalar.copy(den[qp * 2 + j][:, :], srow[:, :])
                            nc.sync.dma_start(
                                den_scr[h * 4 + qp * 2 + j:
                                        h * 4 + qp * 2 + j + 1, :],
                                den[qp * 2 + j][:, :])
                            # drain AV psum (unscaled; scaled after attention)
                            if j == 0:
                                nc.scalar.copy(o_t[h][:, s0:s0 + 512],
                                               po[j][:, :])
                            else:
                                nc.vector.tensor_copy(o_t[h][:, s0:s0 + 512],
                                                      po[j][:, :])
                # batched deferred scale: one reciprocal + one gate multiply
                # for all 20 (head, col-block) rows, then per-block broadcasts
                dsb = asb.tile([HPC * 4, 512], F32, name="dsb")
                nc.sync.dma_start(dsb[:, :], den_scr[:, :])
                rca = asb.tile([HPC * 4, 512], F32, name="rca")
                nc.vector.reciprocal(rca[:, :], dsb[:, :])
                gsb = asb.tile([HPC * 4, 512], F32, name="gsb")
                nc.sync.dma_start(
                    gsb[:, :],
                    gate_scr[:, :].rearrange("h (b c) -> (h b) c", b=4))
                nc.vector.tensor_mul(rca[:, :], rca[:, :], gsb[:, :])
                nc.sync.dma_start(
                    scale_scr[:, :].rearrange("h (b c) -> (h b) c", b=4),
                    rca[:, :])
                # apply to o_t, block-major so projection can chase
                for blk in range(4):
                    s0 = blk * 512
                    for h in range(HPC):
                        sb = asb.tile([128, 512], F32, name="scaleb")
                        nc.sync.dma_start(
                            sb[:, :],
                            scale_scr[h:h + 1, s0:s0 + 512].to_broadcast(
                                (128, 512)))
                        nc.vector.tensor_mul(o_t[h][:, s0:s0 + 512],
                                             o_t[h][:, s0:s0 + 512], sb[:, :])

            if opt.get("debug"):
                nc.sync.dma_start(dbg["q0"][:, :], q_t[0][:, :].bitcast(F32))
                nc.sync.dma_start(dbg["k"][:, :], k_t[:, :].bitcast(F32))
                nc.sync.dma_start(dbg["vnat"][:, :], vnat[:, :])
                nc.sync.dma_start(dbg["nk"][:, :], nk_col[:, :])
                nc.sync.dma_start(dbg["gates"][:, :], gates[:, :])
                nc.sync.dma_start(dbg["scale"][:, :], scale_scr[:, :])
                nc.sync.dma_start(dbg["o0"][:, :], o_t[0][:, :])

            # ============ Phase 3: projection ===========================
            with contextlib.ExitStack() as _ph3:
                ent = _ph3.enter_context
                pPJ = ent(tc.tile_pool(name=f"pj_ps{_rep}", bufs=4, space="PSUM"))
                pjs = ent(tc.tile_pool(name=f"pj_sb{_rep}", bufs=2))
                pjw = ent(tc.tile_pool(name=f"pj_wt{_rep}", bufs=2))
                NTP = HID // 1024
                wt_sets = {0: wt0}

                def emit_wt(ntp):
                    n0 = ntp * 1024
                    wt = pjw.tile([128, HPC, 1024], BF16, name="wt")
                    nc.sync.dma_start(
                        wt[:, :, :],
                        wp[:, :, n0:n0 + 1024].rearrange("h p c -> p h c"))
                    wt_sets[ntp] = wt

                for ntp in range(NTP):
                    n0 = ntp * 1024
                    if ntp + 1 < NTP:
                        emit_wt(ntp + 1)
                    wt = wt_sets.pop(ntp)
                    for st in range(ST):
                        s0 = st * 128
                        ob = pjs.tile([128, 1024], BF16, name="outsb")
                        pp = [pPJ.tile([128, 512], F32, name=f"pj{j}")
                              for j in range(2)]
                        for j in range(2):
                            for h in range(HPC):
                                nc.tensor.matmul(
                                    pp[j][:, :], o_t[h][:, s0:s0 + 128],
                                    wt[:, h, j * 512:(j + 1) * 512],
                                    start=(h == 0), stop=(h == HPC - 1))
                        nc.vector.tensor_copy(ob[:, 0:512], pp[0][:, :])
                        nc.scalar.copy(ob[:, 512:1024], pp[1][:, :])
                        nc.sync.dma_start(out[s0:s0 + 128, n0:n0 + 1024],
                                          ob[:, :])
    nc.finalize()
    return nc


# ---------------- host-side prep & execution ----------------

_CACHE = {}


def _get_exec(repeat=1):
    key = (repeat, tuple(sorted(BUILD_OPTS.items())))
    if key in _CACHE:
        return _CACHE[key]

    import jax
    from concourse import bass2jax, mybir as mb
    from jax.experimental.shard_map import shard_map
    from jax.sharding import Mesh, PartitionSpec

    bass2jax.install_neuronx_cc_hook()
    nc = build_program(repeat)

    part_name = nc.partition_id_tensor.name if nc.partition_id_tensor else None
    in_names, out_names, out_avals = [], [], []
    for alloc in nc.m.functions[0].allocations:
        if not isinstance(alloc, mb.MemoryLocationSet):
            continue
        name = alloc.memorylocations[0].name
        if alloc.kind == "ExternalInput":
            if name != part_name:
                in_names.append(name)
        elif alloc.kind == "ExternalOutput":
            out_names.append(name)
            out_avals.append(jax.core.ShapedArray(tuple(alloc.tensor_shape),
                                                  mb.dt.np(alloc.dtype)))
    n_params = len(in_names)
    all_names = in_names + out_names
    if part_name is not None:
        all_names = all_names + [part_name]

    def _body(*args):
        operands = list(args)
        if part_name is not None:
            operands.append(bass2jax.partition_id_tensor())
        outs = bass2jax._bass_exec_p.bind(
            *operands,
            out_avals=tuple(out_avals),
            in_names=tuple(all_names),
            out_names=tuple(out_names),
            lowering_input_output_aliases=(),
            sim_require_finite=True,
            sim_require_nnan=True,
            nc=nc,
        )
        return tuple(outs)

    devices = jax.devices()[:NC]
    mesh = Mesh(np.asarray(devices), ("core",))
    spec = (PartitionSpec("core"),) * (n_params + len(out_names))
    fn = jax.jit(shard_map(_body, mesh=mesh, in_specs=spec,
                           out_specs=(PartitionSpec("core"),) * len(out_names),
                           check_rep=False), keep_unused=True)
    _CACHE[key] = dict(fn=fn, nc=nc, in_names=in_names, out_names=out_names,
                       out_avals=out_avals, mesh=mesh)
    return _CACHE[key]


def prep_inputs(x, rope_cos, rope_sin, w_pre_norm, w_qkv, w_q_norm, w_k_norm,
                w_proj):
    """Build the per-core input dict list (host-side sharding/layout only)."""
    import ml_dtypes
    bf16 = ml_dtypes.bfloat16
    x = np.asarray(x, np.float32)
    w_qkv = np.asarray(w_qkv, np.float32)
    w_proj = np.asarray(w_proj, np.float32)
    w_pre = np.asarray(w_pre_norm, np.float32)
    w_qn = np.asarray(w_q_norm, np.float32)
    w_kn = np.asarray(w_k_norm, np.float32)
    cos = np.asarray(rope_cos, np.float32)[0]   # [S, D]
    sin = np.asarray(rope_sin, np.float32)[0]

    xT = np.ascontiguousarray(x[0].T).reshape(HT, 128, S).astype(np.float16)

    cosT = np.ascontiguousarray(cos.T)          # [D, S]
    sinT = np.ascontiguousarray(sin.T)
    sign = np.where(np.arange(D) < D // 2, -1.0, 1.0).astype(np.float32)

    def rope_tables(w):
        w_swap = np.concatenate([w[D // 2:], w[:D // 2]])
        c = cosT * w[:, None]
        s = sinT * (sign * w_swap)[:, None]
        return (np.ascontiguousarray(c).astype(np.float16),
                np.ascontiguousarray(s).astype(np.float16))

    cq, sq_ = rope_tables(w_qn)
    ck, sk = rope_tables(w_kn)

    wqkv_eff = w_pre[:, None] * w_qkv           # fold pre-norm weight (exact)
    q_dim, k_dim = NQ * D, NKV * D
    ones = np.ones((128, 1), np.float32)
    ident = np.eye(128, dtype=np.float32)

    in_maps = []
    for c in range(NC):
        wslice = np.concatenate([
            wqkv_eff[:, (HPC * c) * D:(HPC * c + HPC) * D],
            wqkv_eff[:, q_dim + c * D:q_dim + (c + 1) * D],
            wqkv_eff[:, q_dim + k_dim + c * D:q_dim + k_dim + (c + 1) * D],
            wqkv_eff[:, q_dim + 2 * k_dim + HPC * c:q_dim + 2 * k_dim + HPC * (c + 1)],
        ], axis=1)                               # [HID, 901]
        wslice = np.ascontiguousarray(wslice).reshape(
            HT, 128, QKV_COLS).astype(np.float16)
        wpslice = np.ascontiguousarray(
            w_proj[(HPC * c) * D:(HPC * c + HPC) * D, :]).reshape(
            HPC, 128, HID).astype(bf16)
        in_maps.append({
            "xT": xT, "wq": wslice, "wp": wpslice,
            "cosq": cq, "sinq": sq_, "cosk": ck, "sink": sk,
            "ones_col": ones, "ident": ident,
        })
    return in_maps


def run_in_maps(in_maps):
    """Execute the SPMD program; returns list of per-core {out: [S, HID]}."""
    cache = _get_exec()
    fn, in_names, out_names, out_avals = (cache["fn"], cache["in_names"],
                                          cache["out_names"], cache["out_avals"])
    concat_in = [np.concatenate([m[nm] for m in in_maps], axis=0)
                 for nm in in_names]
    zeros = [np.zeros((NC * a.shape[0], *a.shape[1:]), a.dtype) for a in out_avals]
    outs = fn(*concat_in, *zeros)
    res = []
    for c in range(NC):
        d = {}
        for i, nm in enumerate(out_names):
            shp = out_avals[i].shape
            d[nm] = np.asarray(outs[i]).reshape(NC, *shp)[c]
        res.append(d)
    return res


def kernel(**inputs):
    in_maps = prep_inputs(**inputs)
    res = run_in_maps(in_maps)
    total = res[0]["out"].astype(np.float32)
    for c in range(1, NC):
        total = total + res[c]["out"].astype(np.float32)
    return total.reshape(1, S, HID)


# revision 57
# speedup vs baseline: 3.5444x; 3.5007x over previous
"""Trainium2 Bass kernel for a GQA attention block (RMSNorm -> QKV+gate ->
Q/K-norm -> RoPE -> attention -> gated out -> proj), tensor-parallel over
heads across 8 NeuronCores.

Sharding: core c owns q heads [5c, 5c+5) and kv group c (NQ=40, NKV=8).
Each core computes a partial projection output; partials are summed on host
(row-parallel proj unshard).

Perf structure (v2):
  - fp16 inputs/probabilities/v/o/out (f32 PSUM accumulation), q/k in f32r.
  - QKV phase split into two psum pass-groups per 512-col chunk so drains
    overlap matmuls (PE never waits on psum banks).
  - norms/rope/stats/v-transposes for chunk c emitted during chunk c+1 so
    the attention phase starts with everything roped and runs heads
    back-to-back on the PE.
  - softmax exp has a -ln(16) bias folded in (cancels in the ratio) so
    fp16 denominators can't overflow.
  - deferred per-head output scaling (gate * 1/denom) executes during the
    next head; projection in fp16 with prefetched weights.
"""
import contextlib
import math
import sys

sys.path.insert(0, "/opt/trn_rl_repo")

import numpy as np

import bass_rust as _bass_rust

import concourse.bacc as bacc
import concourse.tile as tile
from concourse import mybir
from concourse.hw_specs import get_activation_tables


class _Bacc(bacc.Bacc):
    """Bacc with activation-table choice restricted to the exp+ln set.

    The stock insert_act_table_loads pass picks the FIRST act_func_set
    containing each function, so alternating Ln/Exp activations thrash
    between two tables.  Emptying all sets except `natural_log_exp_and_others`
    (square/copy/exp/ln) and `sigmoid_and_others` forces one resident table
    for the whole kernel (plus a single swap around the sigmoid).
    """

    _KEEP_SETS = {"natural_log_exp_and_others", "sigmoid_and_others"}

    def insert_act_table_loads(self):
        has_activation = any(
            isinstance(i, mybir.InstActivation)
            for b in self.main_func.blocks
            for i in b.instructions
        )
        if not has_activation:
            return
        tables = [
            (name, (fns if name in self._KEEP_SETS else set()))
            for name, fns in get_activation_tables(self.m.arch).items()
        ]
        _bass_rust.insert_act_table_loads(self, tables)

NQ, NKV, D, HID = 40, 8, 128, 5120
S = 2048
NC = 8
HPC = NQ // NC          # q heads per core = 5
EPS = 1e-6
HT = HID // 128         # 40 hid tiles
ST = S // 128           # 16 seq tiles of 128
NCH = S // 512          # 4 chunks of 512
KT = S // 128           # 16 k-tiles
QKV_COLS = HPC * D + 2 * D + HPC   # 901
NLN16 = -math.log(16.0)            # exp bias so fp16 sums can't overflow
F32 = mybir.dt.float32
F32R = mybir.dt.float32r
F16 = mybir.dt.float16
BF16 = mybir.dt.bfloat16
AF = mybir.ActivationFunctionType
BUILD_OPTS = {}


def build_program(repeat=1):
    opt = BUILD_OPTS
    nc = _Bacc(None, target_bir_lowering=False)

    # register activation-bias constants (mirrors Bass.__init__ registration)
    for val in (EPS, float(D) * EPS, NLN16):
        t = nc.alloc_sbuf_tensor(f"const-float32-{val}", [128, 1], F32)
        nc.gpsimd.memset(t.ap(), val)
        nc.const_aps.aps[(F32, val)] = t.ap()
    nc.all_engine_barrier()

    # ---- I/O ----
    xT = nc.dram_tensor("xT", [HT, 128, S], F16, kind="ExternalInput")
    wq = nc.dram_tensor("wq", [HT, 128, QKV_COLS], F16, kind="ExternalInput")
    wp = nc.dram_tensor("wp", [HPC, 128, HID], BF16, kind="ExternalInput")
    cosq = nc.dram_tensor("cosq", [128, S], F16, kind="ExternalInput")
    sinq = nc.dram_tensor("sinq", [128, S], F16, kind="ExternalInput")
    cosk = nc.dram_tensor("cosk", [128, S], F16, kind="ExternalInput")
    sink = nc.dram_tensor("sink", [128, S], F16, kind="ExternalInput")
    ones_col = nc.dram_tensor("ones_col", [128, 1], F32R, kind="ExternalInput")
    ident = nc.dram_tensor("ident", [128, 128], F32R, kind="ExternalInput")
    out = nc.dram_tensor("out", [S, HID], BF16, kind="ExternalOutput")
    dbg = {}
    if opt.get("debug"):
        dbg["q0"] = nc.dram_tensor("dbg_q0", [128, S], F32, kind="ExternalOutput")
        dbg["k"] = nc.dram_tensor("dbg_k", [128, S], F32, kind="ExternalOutput")
        dbg["vnat"] = nc.dram_tensor("dbg_vnat", [128, S], BF16,
                                     kind="ExternalOutput")
        dbg["gates"] = nc.dram_tensor("dbg_gates", [HPC, S], F32,
                                      kind="ExternalOutput")
        dbg["scale"] = nc.dram_tensor("dbg_scale", [HPC, S], F32,
                                      kind="ExternalOutput")
        dbg["o0"] = nc.dram_tensor("dbg_o0", [128, S], BF16,
                                   kind="ExternalOutput")

    with tile.TileContext(nc, pool_alloc_mode=opt.get("palloc", "stack")) as tc:
      for _rep in range(repeat):
        with tc.tile_pool(name=f"persist{_rep}", bufs=1) as pers, \
             tc.tile_pool(name=f"cols{_rep}", bufs=1) as cols, \
             tc.tile_pool(name=f"scr{_rep}", bufs=1, space="DRAM") as dscr:
            # DRAM row bounces (for partition-broadcast / row->col reshape)
            lnm_scr = dscr.tile([1, S], F32, name="lnm_scr")
            nk_scr = dscr.tile([1, S], F32, name="nk_scr")
            den_scr = dscr.tile([HPC * 4, 512], F32, name="den_scr")
            rrow_scr = dscr.tile([1, S], F32, name="rrow_scr")
            gate_scr = dscr.tile([HPC, S], F32, name="gate_scr")
            nq_scr = dscr.tile([HPC, S], F32, name="nq_scr")
            scale_scr = dscr.tile([HPC, S], F32, name="scale_scr")
            # persistent small tiles
            t_ones = cols.tile([128, 1], F32R, name="ones")
            nc.sync.dma_start(t_ones[:, :], ones_col[:, :])
            t_ones16 = cols.tile([128, 1], F16, name="ones16")
            nc.gpsimd.memset(t_ones16[:, :], 1.0)
            t_onesb = cols.tile([128, 1], BF16, name="onesb")
            nc.gpsimd.memset(t_onesb[:, :], 1.0)
            t_id = cols.tile([128, 128], F32R, name="ident")
            nc.sync.dma_start(t_id[:, :], ident[:, :])
            # rope tables resident (fp16); DMAs are emitted after chunk 0's
            # first loads (tables are first needed during chunk 1)
            tcq = cols.tile([128, S], F16, name="tcq")
            tsq = cols.tile([128, S], F16, name="tsq")
            tck = cols.tile([128, S], F16, name="tck")
            tsk = cols.tile([128, S], F16, name="tsk")

            q_t = [pers.tile([128, S], F32R, name=f"q{h}") for h in range(HPC)]
            k_t = pers.tile([128, S], F32R, name="kT")
            vnat = pers.tile([128, S], BF16, name="vnat")
            o_t = [pers.tile([128, S], BF16, name=f"o{h}") for h in range(HPC)]
            gates = pers.tile([HPC, S], F32, name="gates")
            r_col = cols.tile([128, KT], F32, name="r_col")
            # proj ntp0 weights, prefetched during attention
            wt0 = pers.tile([128, HPC, 1024], BF16, name="wt0")

            # ============ Phase 1: QKV (+ fused stats/rope/transposes) =====
            NQD = 10  # hid quad-tiles per chunk pass
            with contextlib.ExitStack() as _ph1:
                ent = _ph1.enter_context
                psA = ent(tc.tile_pool(name=f"psA{_rep}", bufs=1, space="PSUM"))
                psB = ent(tc.tile_pool(name=f"psB{_rep}", bufs=1, space="PSUM"))
                psS = ent(tc.tile_pool(name=f"psS{_rep}", bufs=1, space="PSUM"))
                xtp = ent(tc.tile_pool(name=f"xt{_rep}", bufs=4))
                xtpb = ent(tc.tile_pool(name=f"xtb{_rep}", bufs=3))
                wsap = ent(tc.tile_pool(name=f"wsa{_rep}", bufs=4))
                wsbp = ent(tc.tile_pool(name=f"wsb{_rep}", bufs=2))
                sqbig = ent(tc.tile_pool(name=f"sq{_rep}", bufs=2))
                sqsml = ent(tc.tile_pool(name=f"sqs{_rep}", bufs=2))
                accp = ent(tc.tile_pool(name=f"accx{_rep}", bufs=1))
                accrp = ent(tc.tile_pool(name=f"accr{_rep}", bufs=2))
                vstp = ent(tc.tile_pool(name=f"vst{_rep}", bufs=2))
                rowp = ent(tc.tile_pool(name=f"row{_rep}", bufs=1))
                ropep = ent(tc.tile_pool(name=f"rope{_rep}", bufs=2))

                def emit_stats(ch):
                    """Partition-sum stats for chunk ch (runs during ch+1).

                    Sequential through the psS bank: pre-norm row, k row,
                    q rows.  Emits the DRAM bounces the rope/gate/exp-scale
                    consumers read back.
                    """
                    c0 = ch * 512
                    # pre-norm: lnm = ln(mean_hid x^2 + eps); accr folded by
                    # the chunk-ch pass-B code into rowp tile (returned there)
                    accr = stats_accr.pop(ch)
                    pr = psS.tile([1, 512], F32, name="sm")
                    nc.tensor.matmul(pr[:, :], t_ones16[:, :], accr[:, :],
                                     start=True, stop=True)
                    lnm_row = rowp.tile([1, 512], F32, name="lnm_row")
                    nc.scalar.activation(lnm_row[:, :], pr[:, :], AF.Ln,
                                         bias=EPS, scale=1.0 / HID)
                    nc.sync.dma_start(lnm_scr[0:1, c0:c0 + 512], lnm_row[:, :])
                    r_row = rowp.tile([1, 512], F32, name="r_row")
                    nc.scalar.activation(r_row[:, :], lnm_row[:, :], AF.Exp,
                                         bias=0.0, scale=-0.5)
                    nc.sync.dma_start(rrow_scr[0:1, c0:c0 + 512], r_row[:, :])
                    # r as columns for the v fold
                    lcp = rowp.tile([128, 4], F32, name="lcp")
                    nc.sync.dma_start(
                        lcp[:, :],
                        lnm_scr[0, c0:c0 + 512].rearrange("(t p) -> p t", p=128))
                    nc.scalar.activation(r_col[:, ch * 4:ch * 4 + 4], lcp[:, :],
                                         AF.Exp, bias=0.0, scale=-0.5)
                    # k-norm stats
                    ksq = sqsml.tile([128, 512], F16, name="sqc")
                    nc.scalar.activation(ksq[:, :],
                                         k_t[:, c0:c0 + 512].bitcast(F32),
                                         AF.Square)
                    pn = psS.tile([1, 512], F32, name="sm")
                    nc.tensor.matmul(pn[:, :], t_ones16[:, :], ksq[:, :],
                                     start=True, stop=True)
                    lnk_row = rowp.tile([1, 512], F32, name="lnk_row")
                    nc.scalar.activation(lnk_row[:, :], pn[:, :], AF.Ln,
                                         bias=D * EPS, scale=1.0)
                    nkr = rowp.tile([1, 512], F32, name="nkr")
                    nc.scalar.activation(nkr[:, :], lnk_row[:, :], AF.Exp,
                                         bias=0.0, scale=-0.5)
                    nc.sync.dma_start(nk_scr[0:1, c0:c0 + 512], nkr[:, :])
                    # q-norm stats per head -> nq rows in DRAM
                    for h in range(HPC):
                        qsq = sqsml.tile([128, 512], F16, name="sqc")
                        nc.scalar.activation(qsq[:, :],
                                             q_t[h][:, c0:c0 + 512].bitcast(F32),
                                             AF.Square)
                        pq = psS.tile([1, 512], F32, name="sm")
                        nc.tensor.matmul(pq[:, :], t_ones16[:, :], qsq[:, :],
                                         start=True, stop=True)
                        lnq = rowp.tile([1, 512], F32, name="lnq")
                        nc.scalar.activation(lnq[:, :], pq[:, :], AF.Ln,
                                             bias=EPS, scale=1.0 / D)
                        nqr = rowp.tile([1, 512], F32, name="nqr")
                        nc.scalar.activation(nqr[:, :], lnq[:, :], AF.Exp,
                                             bias=0.0, scale=-0.5)
                        nc.sync.dma_start(nq_scr[h:h + 1, c0:c0 + 512],
                                          nqr[:, :])

                def emit_vtrans(ch):
                    """v transposes for chunk ch -> vnat (fp16, r folded)."""
                    vstg = v_stg.pop(ch)
                    for j in range(4):
                        kt = ch * 4 + j
                        ptr = psS.tile([128, 128], F32R, name="sm")
                        nc.tensor.transpose(ptr[:, :],
                                            vstg[:, j * 128:(j + 1) * 128],
                                            t_id[:, :])
                        nc.vector.tensor_scalar_mul(
                            vnat[:, kt * 128:(kt + 1) * 128], ptr[:, :],
                            r_col[:, kt:kt + 1])

                def emit_rope(ch, heads=None, do_k=True):
                    """RoPE (+q-norm fold) for chunk ch, in place."""
                    c0 = ch * 512
                    if do_k:
                        # k-norm factor folded here so the softmax exp has a
                        # constant scale (enables paired-kt exps)
                        nkb = ropep.tile([128, 512], F32, name="nb")
                        nc.sync.dma_start(
                            nkb[:, :],
                            nk_scr[0:1, c0:c0 + 512].to_broadcast((128, 512)))
                        rot = ropep.tile([128, 512], F32, name="rot")
                        nc.sync.dma_start(rot[0:64, :],
                                          k_t[64:128, c0:c0 + 512].bitcast(F32))
                        nc.sync.dma_start(rot[64:128, :],
                                          k_t[0:64, c0:c0 + 512].bitcast(F32))
                        t1 = ropep.tile([128, 512], F32, name="t1")
                        nc.vector.tensor_mul(t1[:, :],
                                             k_t[:, c0:c0 + 512].bitcast(F32),
                                             tck[:, c0:c0 + 512])
                        nc.vector.tensor_mul(rot[:, :], rot[:, :],
                                             tsk[:, c0:c0 + 512])
                        nc.vector.tensor_add(t1[:, :], t1[:, :], rot[:, :])
                        nc.vector.tensor_mul(k_t[:, c0:c0 + 512], t1[:, :],
                                             nkb[:, :])
                    for h in (range(HPC) if heads is None else heads):
                        nb = ropep.tile([128, 512], F32, name="nb")
                        nc.sync.dma_start(
                            nb[:, :],
                            nq_scr[h:h + 1, c0:c0 + 512].to_broadcast((128, 512)))
                        rot = ropep.tile([128, 512], F32, name="rot")
                        nc.sync.dma_start(rot[0:64, :],
                                          q_t[h][64:128, c0:c0 + 512].bitcast(F32))
                        nc.sync.dma_start(rot[64:128, :],
                                          q_t[h][0:64, c0:c0 + 512].bitcast(F32))
                        t1 = ropep.tile([128, 512], F32, name="t1")
                        nc.vector.tensor_mul(t1[:, :],
                                             q_t[h][:, c0:c0 + 512].bitcast(F32),
                                             tcq[:, c0:c0 + 512])
                        nc.vector.tensor_mul(rot[:, :], rot[:, :],
                                             tsq[:, c0:c0 + 512])
                        nc.vector.tensor_add(t1[:, :], t1[:, :], rot[:, :])
                        nc.vector.tensor_mul(q_t[h][:, c0:c0 + 512], t1[:, :],
                                             nb[:, :])

                stats_accr = {}
                v_stg = {}
                pre_a = {}

                def emit_passA_dma(ch, hq):
                    c0 = ch * 512
                    ht = 4 * hq
                    xt = xtp.tile([128, 4, 512], F16, name="xt")
                    nc.sync.dma_start(
                        xt[:, :, :],
                        xT[ht:ht + 4, :, c0:c0 + 512].rearrange("t p c -> p t c"))
                    ws = wsap.tile([128, 4, 512], F16, name="wsa")
                    nc.sync.dma_start(
                        ws[:, :, :],
                        wq[ht:ht + 4, :, 0:512].rearrange("t p c -> p t c"))
                    return (xt, ws)

                for ch in range(NCH):
                    c0 = ch * 512
                    # ---------------- pass A: q heads 0..3 ----------------
                    pmA = [psA.tile([128, 512], F32, name=f"a{m}")
                           for m in range(4)]
                    accx = accp.tile([128, 2048], F16, name="accx")
                    for hq in range(NQD):
                        ht = 4 * hq
                        if (ch, hq) in pre_a:
                            xt, ws = pre_a.pop((ch, hq))
                        else:
                            xt, ws = emit_passA_dma(ch, hq)
                        for i in range(4):
                            for m in range(4):
                                nc.tensor.matmul(
                                    pmA[m][:, :], ws[:, i, m * 128:(m + 1) * 128],
                                    xt[:, i, :], start=(hq == 0 and i == 0),
                                    stop=(hq == NQD - 1 and i == 3))
                        # x^2 accumulation for the pre-norm (fp16, gpsimd)
                        sq = sqbig.tile([128, 2048], F16, name="sq")
                        nc.scalar.activation(
                            sq[:, :],
                            xt[:, :, :].rearrange("p t c -> p (t c)"),
                            AF.Square)
                        if hq == 0:
                            nc.gpsimd.tensor_copy(accx[:, :], sq[:, :])
                        else:
                            nc.gpsimd.tensor_add(accx[:, :], accx[:, :],
                                                 sq[:, :])
                        # interleave previous chunk's post-work between quads
                        if ch == 0:
                            if hq == 1:
                                nc.sync.dma_start(tcq[:, :], cosq[:, :])
                                nc.sync.dma_start(tsq[:, :], sinq[:, :])
                            elif hq == 2:
                                nc.sync.dma_start(tck[:, :], cosk[:, :])
                                nc.sync.dma_start(tsk[:, :], sink[:, :])
                        if ch > 0:
                            if hq == 1:
                                emit_stats(ch - 1)
                            elif hq == 4:
                                emit_vtrans(ch - 1)
                            elif hq == 6:
                                emit_rope(ch - 1)
                    # ---------------- pass B: q4, k, v, gate --------------
                    pmB = [psB.tile([128, 512], F32, name=f"b{m}")
                           for m in range(3)]
                    pg = psS.tile([5, 512], F32, name="sm")
                    for hq in range(NQD):
                        ht = 4 * hq
                        # prefetch next chunk's first pass-A quads ahead of
                        # this pass's slot-blocked DMAs (avoids sync-engine
                        # head-of-line blocking at the chunk boundary)
                        if hq in (0, 1) and ch + 1 < NCH:
                            pre_a[(ch + 1, hq)] = emit_passA_dma(ch + 1, hq)
                        xt2 = xtpb.tile([128, 4, 512], F16, name="xt")
                        nc.sync.dma_start(
                            xt2[:, :, :],
                            xT[ht:ht + 4, :, c0:c0 + 512].rearrange(
                                "t p c -> p t c"))
                        ws = wsbp.tile([128, 4, 389], F16, name="wsb")
                        nc.sync.dma_start(
                            ws[:, :, :],
                            wq[ht:ht + 4, :, 512:901].rearrange("t p c -> p t c"))
                        for i in range(4):
                            for m in range(3):
                                nc.tensor.matmul(
                                    pmB[m][:, :], ws[:, i, m * 128:(m + 1) * 128],
                                    xt2[:, i, :], start=(hq == 0 and i == 0),
                                    stop=(hq == NQD - 1 and i == 3))
                            nc.tensor.matmul(pg[:, :], ws[:, i, 384:389],
                                             xt2[:, i, :],
                                             start=(hq == 0 and i == 0),
                                             stop=(hq == NQD - 1 and i == 3))
                        if hq == 0:
                            # drains of pass A overlap pass-B matmuls
                            nc.vector.tensor_copy(q_t[0][:, c0:c0 + 512],
                                                  pmA[0][:, :])
                            nc.scalar.copy(q_t[1][:, c0:c0 + 512], pmA[1][:, :])
                        elif hq == 1:
                            nc.vector.tensor_copy(q_t[2][:, c0:c0 + 512],
                                                  pmA[2][:, :])
                            nc.scalar.copy(q_t[3][:, c0:c0 + 512], pmA[3][:, :])
                        elif hq == 2:
                            # fold x^2 quad-halves -> accr for emit_stats
                            accr = accrp.tile([128, 512], F16, name="accr")
                            nc.vector.tensor_add(accr[:, :], accx[:, 0:512],
                                                 accx[:, 512:1024])
                            nc.vector.tensor_add(accr[:, :], accr[:, :],
                                                 accx[:, 1024:1536])
                            nc.vector.tensor_add(accr[:, :], accr[:, :],
                                                 accx[:, 1536:2048])
                            stats_accr[ch] = accr
                    # drains of pass B (overlap next chunk's pass A)
                    nc.scalar.copy(q_t[4][:, c0:c0 + 512], pmB[0][:, :])
                    nc.vector.tensor_copy(k_t[:, c0:c0 + 512], pmB[1][:, :])
                    vstg = vstp.tile([128, 512], F32R, name="vstg")
                    nc.vector.tensor_copy(vstg[:, :], pmB[2][:, :])
                    v_stg[ch] = vstg
                    nc.vector.tensor_copy(gates[:, c0:c0 + 512], pg[:, :])

                # tail: post-work for the last chunk; k + q0 first so
                # attention head 0 can start as early as possible
                emit_stats(NCH - 1)
                emit_vtrans(NCH - 1)
                emit_rope(NCH - 1, heads=[0], do_k=True)
                # gates: g = sigmoid(r * g_raw) (one table swap)
                for ch in range(NCH):
                    c0 = ch * 512
                    rb = rowp.tile([5, 512], F32, name="rb")
                    nc.sync.dma_start(
                        rb[:, :],
                        rrow_scr[0:1, c0:c0 + 512].to_broadcast((5, 512)))
                    nc.vector.tensor_mul(gates[:, c0:c0 + 512],
                                         gates[:, c0:c0 + 512], rb[:, :])
                nc.scalar.activation(gates[:, :], gates[:, :], AF.Sigmoid)
                nc.sync.dma_start(gate_scr[:, :], gates[:, :])
                emit_rope(NCH - 1, heads=[1, 2, 3, 4], do_k=False)
                # prefetch proj ntp0 weights (lands during attention)
                nc.sync.dma_start(
                    wt0[:, :, :],
                    wp[:, :, 0:1024].rearrange("h p c -> p h c"))

            # ============ Phase 2: attention ============================
            with contextlib.ExitStack() as _ph2:
                ent = _ph2.enter_context
                pSC = ent(tc.tile_pool(name=f"at_sc{_rep}", bufs=1, space="PSUM"))
                pAV = ent(tc.tile_pool(name=f"at_av{_rep}", bufs=1, space="PSUM"))
                pMisc = ent(tc.tile_pool(name=f"at_ms{_rep}", bufs=2, space="PSUM"))
                etp = ent(tc.tile_pool(name=f"at_et{_rep}", bufs=4))
                acp = ent(tc.tile_pool(name=f"at_ac{_rep}", bufs=2))
                asb = ent(tc.tile_pool(name=f"at_sb{_rep}", bufs=2))
                NPAIR = KT // 2
                # gate rows, prefetched once for the deferred scale; the two
                # batches get separate partition-0 tiles (DVE reads cannot
                # start at partition 16)
                gsb = asb.tile([16, 512], F32, name="gsb")
                nc.sync.dma_start(
                    gsb[:, :],
                    gate_scr[0:4, :].rearrange("h (b c) -> (h b) c", b=4))
                gh4 = asb.tile([4, 512], F32, name="gh4")
                nc.sync.dma_start(
                    gh4[:, :],
                    gate_scr[4:5, :].rearrange("h (b c) -> (h b) c", b=4))
                dsb = asb.tile([16, 512], F32, name="dsb")
                dh4 = asb.tile([4, 512], F32, name="dh4")

                def emit_scale(h0, h1):
                    """Reciprocal+gate+broadcast scale for heads [h0, h1)."""
                    n = (h1 - h0) * 4
                    dt_, gt_ = (dsb, gsb) if h0 == 0 else (dh4, gh4)
                    rca = asb.tile([n, 512], F32, name=f"rca{h0}")
                    nc.vector.reciprocal(rca[:, :], dt_[0:n, :])
                    nc.vector.tensor_mul(rca[:, :], rca[:, :], gt_[0:n, :])
                    nc.sync.dma_start(
                        scale_scr[h0:h1, :].rearrange("h (b c) -> (h b) c", b=4),
                        rca[:, :])
                    for blk in range(4):
                        s0 = blk * 512
                        for h in range(h0, h1):
                            sb = asb.tile([128, 512], F32, name="scaleb")
                            nc.sync.dma_start(
                                sb[:, :],
                                scale_scr[h:h + 1, s0:s0 + 512].to_broadcast(
                                    (128, 512)))
                            nc.vector.tensor_mul(o_t[h][:, s0:s0 + 512],
                                                 o_t[h][:, s0:s0 + 512],
                                                 sb[:, :])

                for h in range(HPC):
                    den = [asb.tile([1, 512], F32, name=f"den{i}")
                           for i in range(4)]
                    for qp in range(2):
                        c0 = qp * 1024
                        po = [pAV.tile([128, 512], F32, name=f"av{j}")
                              for j in range(2)]
                        # 2 denominator chains on vector (bf16, 2x mode)
                        acc = [acp.tile([128, 2048], BF16, name=f"acc{i}")
                               for i in range(2)]
                        ps_tiles = {}

                        def emit_sc(pt):
                            ps = pSC.tile([128, 2048], F32, name="sc")
                            for sub in range(2):
                                k0 = (2 * pt + sub) * 128
                                for j in range(2):
                                    nc.tensor.matmul(
                                        ps[:, sub * 1024 + j * 512:
                                           sub * 1024 + (j + 1) * 512],
                                        k_t[:, k0:k0 + 128],
                                        q_t[h][:, c0 + j * 512:c0 + (j + 1) * 512],
                                        start=True, stop=True)
                            ps_tiles[pt] = ps

                        emit_sc(0)
                        for pt in range(NPAIR):
                            if pt + 1 < NPAIR:
                                emit_sc(pt + 1)
                            ps = ps_tiles.pop(pt)
                            et = etp.tile([128, 2048], BF16, name="et")
                            nc.scalar.activation(et[:, :], ps[:, :], AF.Exp,
                                                 bias=NLN16, scale=1.0)
                            for sub in range(2):
                                k0 = (2 * pt + sub) * 128
                                for j in range(2):
                                    nc.tensor.matmul(
                                        po[j][:, :], vnat[:, k0:k0 + 128],
                                        et[:, sub * 1024 + j * 512:
                                           sub * 1024 + (j + 1) * 512],
                                        start=(pt == 0 and sub == 0),
                                        stop=(pt == NPAIR - 1 and sub == 1))
                            a = pt % 2
                            if pt < 2:
                                nc.vector.tensor_copy(acc[a][:, :], et[:, :])
                            else:
                                nc.vector.tensor_add(acc[a][:, :],
                                                     acc[a][:, :], et[:, :])
                        for j in range(2):
                            s0 = c0 + j * 512
                            srow = pMisc.tile([1, 512], F32, name="srow")
                            idx = 0
                            for a in range(2):
                                for sub in range(2):
                                    nc.tensor.matmul(
                                        srow[:, :], t_onesb[:, :],
                                        acc[a][:, sub * 1024 + j * 512:
                                               sub * 1024 + (j + 1) * 512],
                                        start=(idx == 0), stop=(idx == 3))
                                    idx += 1
                            nc.vector.tensor_copy(den[qp * 2 + j][:, :],
                                                  srow[:, :])
                            if h < HPC - 1:
                                drow = dsb[h * 4 + qp * 2 + j:
                                           h * 4 + qp * 2 + j + 1, :]
                            else:
                                drow = dh4[qp * 2 + j:qp * 2 + j + 1, :]
                            nc.sync.dma_start(drow, den[qp * 2 + j][:, :])
                            # drain AV psum (unscaled; scaled after attention)
                            nc.vector.tensor_copy(o_t[h][:, s0:s0 + 512],
                                                  po[j][:, :])
                    if h == HPC - 2:
                        # heads 0..3 scale runs while the last head computes
                        emit_scale(0, HPC - 1)
                emit_scale(HPC - 1, HPC)

            if opt.get("debug"):
                nc.sync.dma_start(dbg["q0"][:, :], q_t[0][:, :].bitcast(F32))
                nc.sync.dma_start(dbg["k"][:, :], k_t[:, :].bitcast(F32))
                nc.sync.dma_start(dbg["vnat"][:, :], vnat[:, :])
                nc.sync.dma_start(dbg["gates"][:, :], gates[:, :])
                nc.sync.dma_start(dbg["scale"][:, :], scale_scr[:, :])
                nc.sync.dma_start(dbg["o0"][:, :], o_t[0][:, :])

            # ============ Phase 3: projection ===========================
            with contextlib.ExitStack() as _ph3:
                ent = _ph3.enter_context
                pPJ = ent(tc.tile_pool(name=f"pj_ps{_rep}", bufs=4, space="PSUM"))
                pjs = ent(tc.tile_pool(name=f"pj_sb{_rep}", bufs=2))
                pjw = ent(tc.tile_pool(name=f"pj_wt{_rep}", bufs=2))
                NTP = HID // 1024
                wt_sets = {0: wt0}

                def emit_wt(ntp):
                    n0 = ntp * 1024
                    wt = pjw.tile([128, HPC, 1024], BF16, name="wt")
                    nc.sync.dma_start(
                        wt[:, :, :],
                        wp[:, :, n0:n0 + 1024].rearrange("h p c -> p h c"))
                    wt_sets[ntp] = wt

                for ntp in range(NTP):
                    n0 = ntp * 1024
                    if ntp + 1 < NTP:
                        emit_wt(ntp + 1)
                    wt = wt_sets.pop(ntp)
                    for st in range(ST):
                        s0 = st * 128
                        ob = pjs.tile([128, 1024], BF16, name="outsb")
                        pp = [pPJ.tile([128, 512], F32, name=f"pj{j}")
                              for j in range(2)]
                        for j in range(2):
                            for h in range(HPC):
                                nc.tensor.matmul(
                                    pp[j][:, :], o_t[h][:, s0:s0 + 128],
                                    wt[:, h, j * 512:(j + 1) * 512],
                                    start=(h == 0), stop=(h == HPC - 1))
                        nc.vector.tensor_copy(ob[:, 0:512], pp[0][:, :])
                        nc.scalar.copy(ob[:, 512:1024], pp[1][:, :])
                        nc.sync.dma_start(out[s0:s0 + 128, n0:n0 + 1024],
                                          ob[:, :])
    nc.finalize()
    return nc


# ---------------- host-side prep & execution ----------------

_CACHE = {}


def _get_exec(repeat=1):
    key = (repeat, tuple(sorted(BUILD_OPTS.items())))
    if key in _CACHE:
        return _CACHE[key]

    import jax
    from concourse import bass2jax, mybir as mb
    from jax.experimental.shard_map import shard_map
    from jax.sharding import Mesh, PartitionSpec

    bass2jax.install_neuronx_cc_hook()
    nc = build_program(repeat)

    part_name = nc.partition_id_tensor.name if nc.partition_id_tensor else None
    in_names, out_names, out_avals = [], [], []
    for alloc in nc.m.functions[0].allocations:
        if not isinstance(alloc, mb.MemoryLocationSet):
            continue
        name = alloc.memorylocations[0].name
        if alloc.kind == "ExternalInput":
            if name != part_name:
                in_names.append(name)
        elif alloc.kind == "ExternalOutput":
            out_names.append(name)
            out_avals.append(jax.core.ShapedArray(tuple(alloc.tensor_shape),
                                                  mb.dt.np(alloc.dtype)))
    n_params = len(in_names)
    all_names = in_names + out_names
    if part_name is not None:
        all_names = all_names + [part_name]

    def _body(*args):
        operands = list(args)
        if part_name is not None:
            operands.append(bass2jax.partition_id_tensor())
        outs = bass2jax._bass_exec_p.bind(
            *operands,
            out_avals=tuple(out_avals),
            in_names=tuple(all_names),
            out_names=tuple(out_names),
            lowering_input_output_aliases=(),
            sim_require_finite=True,
            sim_require_nnan=True,
            nc=nc,
        )
        return tuple(outs)

    devices = jax.devices()[:NC]
    mesh = Mesh(np.asarray(devices), ("core",))
    spec = (PartitionSpec("core"),) * (n_params + len(out_names))
    fn = jax.jit(shard_map(_body, mesh=mesh, in_specs=spec,
                           out_specs=(PartitionSpec("core"),) * len(out_names),
                           check_rep=False), keep_unused=True)
    _CACHE[key] = dict(fn=fn, nc=nc, in_names=in_names, out_names=out_names,
                       out_avals=out_avals, mesh=mesh)
    return _CACHE[key]


def prep_inputs(x, rope_cos, rope_sin, w_pre_norm, w_qkv, w_q_norm, w_k_norm,
                w_proj):
    """Build the per-core input dict list (host-side sharding/layout only)."""
    import ml_dtypes
    bf16 = ml_dtypes.bfloat16
    x = np.asarray(x, np.float32)
    w_qkv = np.asarray(w_qkv, np.float32)
    w_proj = np.asarray(w_proj, np.float32)
    w_pre = np.asarray(w_pre_norm, np.float32)
    w_qn = np.asarray(w_q_norm, np.float32)
    w_kn = np.asarray(w_k_norm, np.float32)
    cos = np.asarray(rope_cos, np.float32)[0]   # [S, D]
    sin = np.asarray(rope_sin, np.float32)[0]

    xT = np.ascontiguousarray(x[0].T).reshape(HT, 128, S).astype(np.float16)

    cosT = np.ascontiguousarray(cos.T)          # [D, S]
    sinT = np.ascontiguousarray(sin.T)
    sign = np.where(np.arange(D) < D // 2, -1.0, 1.0).astype(np.float32)

    def rope_tables(w):
        w_swap = np.concatenate([w[D // 2:], w[:D // 2]])
        c = cosT * w[:, None]
        s = sinT * (sign * w_swap)[:, None]
        return (np.ascontiguousarray(c).astype(np.float16),
                np.ascontiguousarray(s).astype(np.float16))

    cq, sq_ = rope_tables(w_qn)
    ck, sk = rope_tables(w_kn)

    wqkv_eff = w_pre[:, None] * w_qkv           # fold pre-norm weight (exact)
    q_dim, k_dim = NQ * D, NKV * D
    ones = np.ones((128, 1), np.float32)
    ident = np.eye(128, dtype=np.float32)

    in_maps = []
    for c in range(NC):
        wslice = np.concatenate([
            wqkv_eff[:, (HPC * c) * D:(HPC * c + HPC) * D],
            wqkv_eff[:, q_dim + c * D:q_dim + (c + 1) * D],
            wqkv_eff[:, q_dim + k_dim + c * D:q_dim + k_dim + (c + 1) * D],
            wqkv_eff[:, q_dim + 2 * k_dim + HPC * c:q_dim + 2 * k_dim + HPC * (c + 1)],
        ], axis=1)                               # [HID, 901]
        wslice = np.ascontiguousarray(wslice).reshape(
            HT, 128, QKV_COLS).astype(np.float16)
        wpslice = np.ascontiguousarray(
            w_proj[(HPC * c) * D:(HPC * c + HPC) * D, :]).reshape(
            HPC, 128, HID).astype(bf16)
        in_maps.append({
            "xT": xT, "wq": wslice, "wp": wpslice,
            "cosq": cq, "sinq": sq_, "cosk": ck, "sink": sk,
            "ones_col": ones, "ident": ident,
        })
    return in_maps


def run_in_maps(in_maps):
    """Execute the SPMD program; returns list of per-core {out: [S, HID]}."""
    cache = _get_exec()
    fn, in_names, out_names, out_avals = (cache["fn"], cache["in_names"],
                                          cache["out_names"], cache["out_avals"])
    concat_in = [np.concatenate([m[nm] for m in in_maps], axis=0)
                 for nm in in_names]
    zeros = [np.zeros((NC * a.shape[0], *a.shape[1:]), a.dtype) for a in out_avals]
    outs = fn(*concat_in, *zeros)
    res = []
    for c in range(NC):
        d = {}
        for i, nm in enumerate(out_names):
            shp = out_avals[i].shape
            d[nm] = np.asarray(outs[i]).reshape(NC, *shp)[c]
        res.append(d)
    return res


def kernel(**inputs):
    in_maps = prep_inputs(**inputs)
    res = run_in_maps(in_maps)
    total = res[0]["out"].astype(np.float32)
    for c in range(1, NC):
        total = total + res[c]["out"].astype(np.float32)
    return total.reshape(1, S, HID)
